# revision 1
# baseline (speedup 1.0000x reference)
"""AttentionDecoder Trainium2 kernel: 8-way model-parallel LSTM+attention decoder.

Strategy:
  - Weights sharded 8 ways over the gate/hidden dims, SBUF-resident.
  - Activations feature-major [feat, batch]; matmuls are activation-stationary
    (lhsT = activation [K=feat, M=batch], rhs = weight.T [K=feat, N=out_feats]).
  - Per timestep: 3 AllGathers (h0, h1, o) across the 8 cores.
  - Attention refactored: M1T[b] = (h_enc[b] @ W1).T and M2[b] = h_enc[b] @ W2v.T
    are precomputed (hoists h_enc out of the sequential loop), so per step
    scores[b] = M1T[b].T @ h1[:,b] + c1[b] and
    z[b] = a[b] @ M2[b] + W2h @ h1[:,b] + b2,  o = tanh(z).
  - Per-core batch shard for attention: core k owns batch 8k..8k+7.
"""

import os
import warnings

warnings.filterwarnings("ignore")

import numpy as np

VOCAB, E, H, L, B, T, S, V = 32000, 512, 1024, 2, 64, 64, 128, 1024
NCORES = 8
P = 128
BG = B // NCORES  # 8 batch per core for attention
HC = H // NCORES  # 128 hidden feats per core
GC = 4 * HC  # 512 gate rows per core

REMOTE_MODE = int(os.environ.get("DEC_REMOTE", "0"))
USE_REMOTE = REMOTE_MODE >= 1

_COMPILED = None


def _build(n_steps: int):
    import concourse.bass as bass
    import concourse.bacc as bacc
    import concourse.mybir as mybir
    import concourse.tile as tile
    from concourse import masks

    fp32 = mybir.dt.float32
    AF = mybir.ActivationFunctionType
    AX = mybir.AxisListType

    nc = bacc.Bacc(
        "TRN2",
        target_bir_lowering=False,
        debug=False,
        num_devices=NCORES,
        monotonic_sem_count=12,
    )
    rsems = [nc.monotonic_semaphore(i).sem() for i in range(6)]  # h0e,h0o,h1e,h1o,oe,oo
    lsems = [nc.monotonic_semaphore(6 + i).sem() for i in range(6)]
    import os as _os
    _rm = int(_os.environ.get("DEC_REMOTE", "0"))
    RD = [(0, d) for d in range(NCORES)]
    if _rm == 2:
        RD = [None] + [(0, d) for d in range(1, NCORES)]
    RSEM_PER_ROUND = 14 if _rm == 2 else 16

    # ---- DRAM parameters (per-core data) ----
    d_xseq = nc.dram_tensor("xseq", [n_steps, P, 4, B], fp32, kind="ExternalInput")
    d_w0T = nc.dram_tensor("w0T", [16, P, GC], fp32, kind="ExternalInput")
    d_w1T = nc.dram_tensor("w1T", [16, P, GC], fp32, kind="ExternalInput")
    d_b0 = nc.dram_tensor("b0", [B, GC], fp32, kind="ExternalInput")
    d_b1 = nc.dram_tensor("b1", [B, GC], fp32, kind="ExternalInput")
    d_m1t = nc.dram_tensor("m1t", [BG, 8, P, S], fp32, kind="ExternalInput")
    d_c1t = nc.dram_tensor("c1t", [2, P, S], fp32, kind="ExternalInput")
    d_m2s = nc.dram_tensor("m2s", [BG, P, E], fp32, kind="ExternalInput")
    d_w2hT = nc.dram_tensor("w2hT", [8, P, E], fp32, kind="ExternalInput")
    d_b2 = nc.dram_tensor("b2", [BG, E], fp32, kind="ExternalInput")
    d_h0i = nc.dram_tensor("h0i", [P, 8, B], fp32, kind="ExternalInput")
    d_h1i = nc.dram_tensor("h1i", [P, 8, B], fp32, kind="ExternalInput")
    d_oi = nc.dram_tensor("oi", [P, NCORES * 4 * BG], fp32, kind="ExternalInput")
    d_out = nc.dram_tensor("out", [n_steps, BG, E], fp32, kind="ExternalOutput")

    with tile.TileContext(nc) as tc:
        import contextlib

        ctx = contextlib.ExitStack()
        with ctx:
            wpool = ctx.enter_context(tc.tile_pool(name="weights", bufs=1))
            spool = ctx.enter_context(tc.tile_pool(name="state", bufs=1))
            xpool = ctx.enter_context(tc.tile_pool(name="x", bufs=2))
            tpool = ctx.enter_context(tc.tile_pool(name="tmp", bufs=2))
            ppool = ctx.enter_context(tc.tile_pool(name="psum", bufs=2, space="PSUM"))
            p1pool = ctx.enter_context(tc.tile_pool(name="psum1", bufs=1, space="PSUM"))
            dpool = ctx.enter_context(tc.tile_pool(name="dram", bufs=2, space="DRAM"))

            # ---- persistent SBUF tiles ----
            w0T = wpool.tile([P, 16, GC], fp32, tag="w0T")
            w1T = wpool.tile([P, 16, GC], fp32, tag="w1T")
            b0 = wpool.tile([B, GC], fp32, tag="b0")
            b1 = wpool.tile([B, GC], fp32, tag="b1")
            m1t = wpool.tile([P, BG, 8, S], fp32, tag="m1t")
            c1t = wpool.tile([P, 2, S], fp32, tag="c1t")
            m2s = wpool.tile([P, BG, E], fp32, tag="m2s")
            w2hT = wpool.tile([P, 8, E], fp32, tag="w2hT")
            b2 = wpool.tile([BG, E], fp32, tag="b2")
            ident = wpool.tile([P, P], fp32, tag="ident")

            h0f = [spool.tile([P, NCORES * B], fp32, tag=f"h0f{i}", name=f"h0f{i}") for i in range(2)]
            h1f = [spool.tile([P, NCORES * B], fp32, tag=f"h1f{i}", name=f"h1f{i}") for i in range(2)]
            of = [spool.tile([P, NCORES * 4 * BG], fp32, tag=f"of{i}", name=f"of{i}") for i in range(2)]
            c0 = spool.tile([B, HC], fp32, tag="c0")  # cell states, batch-major
            c1 = spool.tile([B, HC], fp32, tag="c1")
            h1my = spool.tile([P, 8, BG], fp32, tag="h1my")

            # ---- prologue loads ----
            nc.sync.dma_start(w0T[:], d_w0T[:].rearrange("kt p g -> p kt g"))
            nc.sync.dma_start(w1T[:], d_w1T[:].rearrange("kt p g -> p kt g"))
            nc.sync.dma_start(b0[:], d_b0[:])
            nc.sync.dma_start(b1[:], d_b1[:])
            nc.scalar.dma_start(m1t[:], d_m1t[:].rearrange("j kt p s -> p j kt s"))
            nc.scalar.dma_start(m2s[:], d_m2s[:].rearrange("j p e -> p j e"))
            nc.sync.dma_start(c1t[:], d_c1t[:].rearrange("h p s -> p h s"))
            nc.scalar.dma_start(w2hT[:], d_w2hT[:].rearrange("kt p e -> p kt e"))
            nc.sync.dma_start(b2[:], d_b2[:])
            masks.make_identity(nc, ident[:])
            nc.sync.dma_start(h0f[1][:].rearrange("p (kc b) -> p kc b", kc=8), d_h0i[:])
            nc.sync.dma_start(h1f[1][:].rearrange("p (kc b) -> p kc b", kc=8), d_h1i[:])
            # of init: slot k holds o columns [c, j] for b-group k
            nc.sync.dma_start(of[1][:], d_oi[:])
            nc.vector.memset(c0[:], 0.0)
            nc.vector.memset(c1[:], 0.0)

            pid = nc.vector.partition_id()
            pid_pl = nc.gpsimd.partition_id()

            def lstm_pointwise(g_sb, cst, h_out):
                """g_sb [B, 4*HC] gate order i,f,g,o; updates cst, writes h_out [B,HC]."""
                gt = tpool.tile([B, HC], fp32, tag="pw_gt")
                ot = tpool.tile([B, HC], fp32, tag="pw_ot")
                ift = tpool.tile([B, 2 * HC], fp32, tag="pw_ift")
                nc.scalar.activation(ift[:], g_sb[:, 0 : 2 * HC], AF.Sigmoid)
                it, ft = ift[:, 0:HC], ift[:, HC : 2 * HC]
                nc.scalar.activation(gt[:], g_sb[:, 2 * HC : 3 * HC], AF.Tanh)
                nc.scalar.activation(ot[:], g_sb[:, 3 * HC : 4 * HC], AF.Sigmoid)
                t1 = tpool.tile([B, HC], fp32, tag="pw_t1")
                nc.vector.tensor_mul(t1[:], ft, cst[:])
                nc.vector.tensor_mul(gt[:], it, gt[:])
                nc.vector.tensor_add(cst[:], t1[:], gt[:])
                tc_ = tpool.tile([B, HC], fp32, tag="pw_tc")
                nc.scalar.activation(tc_[:], cst[:], AF.Tanh)
                nc.vector.tensor_mul(h_out[:], ot[:], tc_[:])

            def evict_src(t, kind, dst_ap, src_ap):
                if t >= 2 and USE_REMOTE:
                    with tc.tile_critical():
                        nc.vector.wait_ge(lsems[2 * kind + (t % 2)], 16 * (t // 2))
                        nc.vector.tensor_copy(dst_ap, src_ap)
                else:
                    nc.vector.tensor_copy(dst_ap, src_ap)

            def exchange(t, kind, src_sb, width, dst_tile):
                """Broadcast my [P,width] chunk into slot pid of everyone's dst_tile."""
                if not USE_REMOTE:
                    bi = dpool.tile([P, width], fp32, tag=f"agi{kind}", name=f"agi{kind}")
                    bo = dpool.tile(
                        [P * NCORES, width], fp32, tag=f"ago{kind}", name=f"ago{kind}"
                    )
                    nc.gpsimd.dma_start(bi[:], src_sb)
                    nc.gpsimd.collective_compute(
                        "AllGather",
                        mybir.AluOpType.bypass,
                        replica_groups=[list(range(NCORES))],
                        ins=[bi.opt()],
                        outs=[bo.opt()],
                    )
                    nc.gpsimd.dma_start(
                        dst_tile[:].rearrange("p (k w) -> p k w", k=NCORES),
                        bo[:].rearrange("(k p) w -> p k w", p=P),
                    )
                    return
                rsem = rsems[2 * kind + (t % 2)]
                nc.gpsimd.remote_dma_broadcast(
                    dst_tile[:, bass.ts(pid_pl, width)],
                    src_sb,
                    rsem,
                    lsems[2 * kind + (t % 2)],
                    rdests=RD,
                )
                nc.gpsimd.trigger_dma(count=None)
                if RSEM_PER_ROUND == 14:
                    # self slot not broadcast; copy locally
                    nc.vector.tensor_copy(
                        dst_tile[:, bass.ts(pid, width)], src_sb
                    )
                with tc.tile_critical():
                    nc.vector.wait_ge(rsem, RSEM_PER_ROUND * (t // 2 + 1))
                    nc.vector.tensor_copy(dst_tile[0:1, 0:1], dst_tile[0:1, 0:1])

            for t in range(n_steps):
                # ---- x load ----
                xt = xpool.tile([P, 4, B], fp32, tag="xt")
                nc.scalar.dma_start(xt[:], d_xseq[t])

                # ---- gates0: K = [x(4) | o(4) | h0(8)] ----
                h0f_r = h0f[(t - 1) % 2]
                h1f_r = h1f[(t - 1) % 2]
                of_r = of[(t - 1) % 2]
                of_rv = of_r[:].rearrange("p (k c j) -> p c k j", k=NCORES, c=4)
                o4 = tpool.tile([P, 4, B], fp32, tag="o4")
                nc.vector.tensor_copy(
                    o4[:].rearrange("p c (k j) -> p c k j", k=NCORES), of_rv
                )
                pg0 = ppool.tile([P, 2, GC], fp32, tag="pg")
                order0 = [0, 1, 2, 3] + [8, 9, 10, 11, 12, 13, 14, 15] + [4, 5, 6, 7]
                for i, kt in enumerate(order0):
                    if kt < 4:
                        lhsT = xt[:, kt, :]
                    elif kt < 8:
                        lhsT = o4[:, kt - 4, :]
                    else:
                        lhsT = h0f_r[:, (kt - 8) * B : (kt - 7) * B]
                    hf = i % 2
                    nc.tensor.matmul(
                        pg0[64 * hf : 64 * hf + 64, hf, :],
                        lhsT,
                        w0T[:, kt, :],
                        start=(i < 2),
                        stop=(i >= 14),
                        tile_position=(0, 64 * hf),
                    )
                g0 = tpool.tile([B, GC], fp32, tag="g0")
                nc.vector.tensor_add(g0[:], pg0[0:64, 0, :], b0[:])
                nc.vector.tensor_add(g0[:], g0[:], pg0[64:128, 1, :])
                h0m = tpool.tile([B, HC], fp32, tag="h0m")
                lstm_pointwise(g0, c0, h0m)

                # ---- transpose h0m -> [HC, B], AG -> h0f ----
                pt0 = p1pool.tile([P, 128], fp32, tag="ptr", name="pt0")
                nc.tensor.transpose(pt0[:, 0:B], h0m[:], ident[0:B, 0:B])
                h0T = tpool.tile([P, B], fp32, tag="h0T")
                evict_src(t, 0, h0T[:], pt0[:, 0:B])
                h0src = h0T[:]

                exchange(t, 0, h0src, B, h0f[t % 2])

                # ---- gates1: K = [h0(8) | h1(8)] ----
                h0f_w = h0f[t % 2]
                pg1 = ppool.tile([P, 2, GC], fp32, tag="pg")
                order1 = [8, 9, 10, 11, 12, 13, 14, 15] + [0, 1, 2, 3, 4, 5, 6, 7]
                for i, kt in enumerate(order1):
                    lhsT = (
                        h0f_w[:, kt * B : (kt + 1) * B]
                        if kt < 8
                        else h1f_r[:, (kt - 8) * B : (kt - 7) * B]
                    )
                    hf = i % 2
                    nc.tensor.matmul(
                        pg1[64 * hf : 64 * hf + 64, hf, :],
                        lhsT,
                        w1T[:, kt, :],
                        start=(i < 2),
                        stop=(i >= 14),
                        tile_position=(0, 64 * hf),
                    )
                g1 = tpool.tile([B, GC], fp32, tag="g1")
                nc.vector.tensor_add(g1[:], pg1[0:64, 0, :], b1[:])
                nc.vector.tensor_add(g1[:], g1[:], pg1[64:128, 1, :])
                h1m = tpool.tile([B, HC], fp32, tag="h1m")
                lstm_pointwise(g1, c1, h1m)

                # ---- transpose h1m, AG -> h1f ----
                pt1 = p1pool.tile([P, 128], fp32, tag="ptr", name="pt1")
                nc.tensor.transpose(pt1[:, 0:B], h1m[:], ident[0:B, 0:B])
                h1T = tpool.tile([P, B], fp32, tag="h1T")
                evict_src(t, 1, h1T[:], pt1[:, 0:B])
                h1src = h1T[:]

                exchange(t, 1, h1src, B, h1f[t % 2])

                # ---- select my batch columns of h1 (query) ----
                h1f_wv = h1f[t % 2][:].rearrange("p (kc b) -> p kc b", kc=8)
                nc.vector.tensor_copy(h1my[:], h1f_wv[:, :, bass.ts(pid, BG)])

                # ---- scores: per-b matvec via tile_position packing ----
                psc = p1pool.tile([P, 2, S], fp32, tag="psc")
                nc.vector.memset(psc[:], 0.0)
                for j in range(BG):
                    half, row = j // 4, 32 * (j % 4)
                    for kt in range(8):
                        nc.tensor.matmul(
                            psc[row : row + 1, half, :],
                            h1my[:, kt, j : j + 1],
                            m1t[:, j, kt, :],
                            start=(kt == 0),
                            stop=(kt == 7),
                            tile_position=(0, row),
                        )
                # ---- softmax over the two halves (garbage rows are fine) ----
                a_sb = tpool.tile([P, 2, S], fp32, tag="a_sb")
                stat = tpool.tile([P, 4], fp32, tag="stat")
                for half in range(2):
                    nc.vector.tensor_add(
                        a_sb[:, half, :], psc[:, half, :], c1t[:, half, :]
                    )
                    nm = stat[:, 2 * half : 2 * half + 1]
                    nc.vector.tensor_reduce(
                        nm, a_sb[:, half, :], axis=AX.X, op=mybir.AluOpType.max,
                        negate=True,
                    )
                    sm = stat[:, 2 * half + 1 : 2 * half + 2]
                    nc.scalar.activation(
                        a_sb[:, half, :], a_sb[:, half, :], AF.Exp, bias=nm,
                        accum_out=sm,
                    )
                    nc.vector.reciprocal(sm, sm)
                    nc.vector.tensor_scalar_mul(a_sb[:, half, :], a_sb[:, half, :], sm)

                # ---- transpose a -> columns; build block-diag lhsT ----
                paT = p1pool.tile([P, 2, S], fp32, tag="psc", name="paT")
                nc.tensor.transpose(paT[:, 0, :], a_sb[:, 0, :], ident[:])
                nc.tensor.transpose(paT[:, 1, :], a_sb[:, 1, :], ident[:])
                abd = tpool.tile([P, BG * BG], fp32, tag="abd")
                nc.vector.memset(abd[:], 0.0)
                # dst cols 9j <- paT cols 128*(j//4) + 32*(j%4), one strided copy
                nc.vector.tensor_copy(
                    abd[:, 0 : BG * BG : 9].rearrange("p (a b) -> p a b", a=2),
                    paT[:].rearrange("p h (c x) -> p h c x", c=4)[:, :, :, 0:1],
                )

                # ---- z = blockdiag(a) @ M2stack + h1my.T @ W2h.T ----
                pz = p1pool.tile([BG, E], fp32, tag="pz")
                for j in range(BG):
                    nc.tensor.matmul(
                        pz[:],
                        abd[:, j * BG : (j + 1) * BG],
                        m2s[:, j, :],
                        start=(j == 0),
                        stop=False,
                    )
                for kt in range(8):
                    nc.tensor.matmul(
                        pz[:], h1my[:, kt, :], w2hT[:, kt, :], start=False,
                        stop=(kt == 7),
                    )
                zt = tpool.tile([BG, E], fp32, tag="zt")
                nc.vector.tensor_add(zt[:], pz[:], b2[:])
                o_sb = tpool.tile([BG, E], fp32, tag="o_sb")
                nc.scalar.activation(o_sb[:], zt[:], AF.Tanh)

                # ---- write output ----
                nc.scalar.dma_start(d_out[t], o_sb[:])

                # ---- transpose o chunks -> [P, 4, BG], AG -> of ----
                poT = p1pool.tile([P, 4, BG], fp32, tag="ptr", name="poT")
                for cchunk in range(4):
                    nc.tensor.transpose(
                        poT[:, cchunk, :],
                        o_sb[:, cchunk * P : (cchunk + 1) * P],
                        ident[0:BG, 0:BG],
                    )
                oT = tpool.tile([P, 4 * BG], fp32, tag="oT")
                evict_src(t, 2, oT[:].rearrange("p (c j) -> p c j", c=4), poT[:])
                osrc = oT[:]

                exchange(t, 2, osrc, 4 * BG, of[t % 2])

    nc.compile()
    return nc


def _host_prep(inputs: dict, n_steps: int):
    """Build per-core in_maps."""
    f32 = np.float32
    tgt = np.asarray(inputs["tgt_batch"])
    h_enc = np.asarray(inputs["h_encoder"], f32)
    emb = np.asarray(inputs["emb"], f32)
    out_init = np.asarray(inputs["output_init"], f32)
    hid_init = np.asarray(inputs["hidden_init"], f32)
    W_ih = np.asarray(inputs["W_ih"], f32)
    W_hh = np.asarray(inputs["W_hh"], f32)
    b_ih = np.asarray(inputs["b_ih"], f32)
    b_hh = np.asarray(inputs["b_hh"], f32)
    W1 = np.asarray(inputs["W1"], f32)
    b1v = np.asarray(inputs["b1"], f32)
    W2 = np.asarray(inputs["W2"], f32)
    b2v = np.asarray(inputs["b2"], f32)

    # x sequence, feature-major, folded [T, P, 4, B]
    xs = emb[tgt[:n_steps]]  # [T, B, E]
    xseq = np.ascontiguousarray(
        xs.transpose(0, 2, 1).reshape(n_steps, 4, P, B).transpose(0, 2, 1, 3)
    )

    # full o / h inits, feature-major folds
    # oi[p, (k, c, j)] = o[c*128+p, 8k+j]
    oi4 = out_init.T.reshape(4, P, NCORES, 8)  # [c, p, k, j]
    oi = np.ascontiguousarray(oi4.transpose(1, 2, 0, 3).reshape(P, NCORES * 4 * 8))
    h0i = np.ascontiguousarray(hid_init[0].T.reshape(8, P, B).transpose(1, 0, 2))
    h1i = np.ascontiguousarray(hid_init[1].T.reshape(8, P, B).transpose(1, 0, 2))

    # attention precompute (host for now; small fraction of FLOPs)
    # M1T[b] = (h_enc[b] @ W1).T  [H, S];  c1[b] = h_enc[b] @ b1  [S]
    # M2[b] = h_enc[b] @ W2v.T  [S, E]
    W2v, W2h = W2[:, :V], W2[:, V:]
    M1T = np.einsum("bsv,vh->bhs", h_enc, W1).astype(f32)  # [B, H, S]
    c1v = np.einsum("bsv,v->bs", h_enc, b1v).astype(f32)  # [B, S]
    M2 = np.einsum("bsv,ev->bse", h_enc, W2v).astype(f32)  # [B, S, E]

    in_maps = []
    for k in range(NCORES):
        rows = np.concatenate([np.arange(g * H + k * HC, g * H + (k + 1) * HC) for g in range(4)])
        W0c = np.concatenate([W_ih[0], W_hh[0]], axis=1)[rows]  # [GC, 2048]
        W1c = np.concatenate([W_ih[1], W_hh[1]], axis=1)[rows]
        w0T = np.ascontiguousarray(W0c.T.reshape(16, P, GC))
        w1T = np.ascontiguousarray(W1c.T.reshape(16, P, GC))
        b0c = np.broadcast_to((b_ih[0] + b_hh[0])[rows], (B, GC)).copy()
        b1c = np.broadcast_to((b_ih[1] + b_hh[1])[rows], (B, GC)).copy()

        bs = np.arange(k * BG, (k + 1) * BG)
        m1tc = np.ascontiguousarray(M1T[bs].reshape(BG, 8, P, S))
        m2sc = np.ascontiguousarray(M2[bs])  # [BG, S, E] (S=P)
        c1tc = np.zeros((2, P, S), f32)
        for j in range(BG):
            c1tc[j // 4, 32 * (j % 4), :] = c1v[bs[j]]
        w2hT = np.ascontiguousarray(W2h.T.reshape(8, P, E))
        b2c = np.broadcast_to(b2v, (BG, E)).copy()

        in_maps.append(
            {
                "xseq": xseq,
                "w0T": w0T,
                "w1T": w1T,
                "b0": b0c,
                "b1": b1c,
                "m1t": m1tc,
                "c1t": c1tc,
                "m2s": m2sc,
                "w2hT": w2hT,
                "b2": b2c,
                "h0i": h0i,
                "h1i": h1i,
                "oi": oi,
            }
        )
    return in_maps


def run(inputs: dict, n_steps: int = T, trace: bool = False):
    global _COMPILED
    from concourse.bass_utils import run_bass_kernel_spmd

    if _COMPILED is None or _COMPILED[1] != n_steps:
        _COMPILED = (_build(n_steps), n_steps)
    nc = _COMPILED[0]
    in_maps = _host_prep(inputs, n_steps)
    res = run_bass_kernel_spmd(
        nc, in_maps, core_ids=list(range(NCORES)), trace=trace
    )
    outs = [res.results[k]["out"] for k in range(NCORES)]  # [T, BG, E] each
    full = np.concatenate(outs, axis=1)  # [T, B, E]
    return np.ascontiguousarray(full.transpose(1, 0, 2)), res  # [B, T, E]


def kernel(**inputs) -> np.ndarray:
    out, _ = run(inputs, T)
    return out.astype(np.float32)



# revision 11
# speedup vs baseline: 7.7528x; 7.7528x over previous
"""AttentionDecoder Trainium2 kernel: 8-way model-parallel LSTM+attention decoder.

v2 — optimized for end-to-end wall clock through the axon tunnel (~82MB/s):
  - Quantized transfer: LSTM weights int8, h_encoder/W1/W2/xseq int16,
    output int16 (~43MB/call vs 210MB fp32 baseline).
  - Attention precompute (M1T = (h_enc @ W1).T, M2 = h_enc @ W2v.T) moved
    from host numpy (was 4.3s) onto the device prologue (fp32 PE matmuls
    on exact int16 operands, scales folded into the PSUM->SBUF copies).
  - Shared tensors (W1, W2, xseq, state inits) shipped as 1/8 shards and
    AllGathered on device.
  - All layout transposes on device (PE transpose / DMA XBAR transpose);
    host prep is quantize + contiguous reshapes only.
  - Steady-state exec path caches the jitted shard_map wrapper (no per-call
    retrace) and creates donated output zeros on device.
  - Step loop matmuls in bf16 (weights already <= 8-bit precision),
    pointwise/softmax in fp32, per-step AllGathers in bf16.

Numerics validated vs reference: rel err ~2.8e-3 (tolerance 2e-2).

Layout summary (per core k):
  - Weights sharded over the 4H gate dim: core k owns gate rows
    [g*H + k*128, g*H + (k+1)*128) for g in 0..3 of both layers.
  - Activations feature-major [feat, batch]; gates matmuls are
    lhsT = activation chunk [K=128 feats, M=B], rhs = weight.T chunk.
  - Per timestep: 3 bf16 AllGathers (h0, h1, o) across the 8 cores.
  - Attention per-core batch shard: core k owns batches 8k..8k+7.
"""

import warnings

warnings.filterwarnings("ignore")

import numpy as np

VOCAB, E, H, L, B, T, S, V = 32000, 512, 1024, 2, 64, 64, 128, 1024
NCORES = 8
P = 128
BG = B // NCORES  # 8 batches per core for attention
HC = H // NCORES  # 128 hidden feats per core
GC = 4 * HC  # 512 gate rows per core

O_SCALE = 32000.0  # fixed output quant scale (tanh output, |o| <= 1)

# ---- AllGather blob layout (int16 elems, per-core contribution) ----
N_W1 = P * H  # [128, 1024] W1 row chunk
N_W2 = 2 * P * E  # [256, 512] W2.T row chunk
N_HI = P * B  # [128, 64] hidden-init feature chunk
N_OI = P * 4 * BG  # [128, 4, 8] output-init chunk
OFF_W1 = 0
OFF_W2 = OFF_W1 + N_W1
OFF_X = OFF_W2 + N_W2


def _blob_layout(n_pad):
    n_x = (n_pad // NCORES) * P * 4 * B
    off_h0 = OFF_X + n_x
    off_h1 = off_h0 + N_HI
    off_oi = off_h1 + N_HI
    nb1 = off_oi + N_OI
    return n_x, off_h0, off_h1, off_oi, nb1


_CACHE = {}


def _build(n_steps: int):
    import concourse.bass as bass
    import concourse.bacc as bacc
    import concourse.mybir as mybir
    import concourse.tile as tile
    from concourse import masks

    fp32 = mybir.dt.float32
    bf16 = mybir.dt.bfloat16
    i16 = mybir.dt.int16
    i8 = mybir.dt.int8
    AF = mybir.ActivationFunctionType
    AX = mybir.AxisListType

    n_pad = ((n_steps + NCORES - 1) // NCORES) * NCORES
    n_x, off_h0, off_h1, off_oi, nb1 = _blob_layout(n_pad)
    x_per_core = (n_pad // NCORES) * P * 4 * B

    nc = bacc.Bacc("TRN2", target_bir_lowering=False, debug=False, num_devices=NCORES)

    # ---- DRAM I/O ----
    d_blob = nc.dram_tensor("blob", [nb1], i16, kind="ExternalInput")
    d_lw = nc.dram_tensor("lw", [L, GC, 2048], i8, kind="ExternalInput")
    d_henc = nc.dram_tensor("henc", [BG, S, V], i16, kind="ExternalInput")
    d_c1r = nc.dram_tensor("c1r", [BG, S], fp32, kind="ExternalInput")
    d_gb = nc.dram_tensor("gb", [3, GC], fp32, kind="ExternalInput")
    d_sc = nc.dram_tensor("sc", [P, 8], fp32, kind="ExternalInput")
    d_out = nc.dram_tensor("out", [n_steps, BG, E], i16, kind="ExternalOutput")

    RG = [list(range(NCORES))]

    with tile.TileContext(nc) as tc:
        import contextlib

        ctx = contextlib.ExitStack()
        with ctx:
            wpool = ctx.enter_context(tc.tile_pool(name="weights", bufs=1))
            spool = ctx.enter_context(tc.tile_pool(name="state", bufs=1))
            propool = ctx.enter_context(tc.tile_pool(name="pro", bufs=1))
            xpool = ctx.enter_context(tc.tile_pool(name="x", bufs=2))
            tpool = ctx.enter_context(tc.tile_pool(name="tmp", bufs=2))
            ppool = ctx.enter_context(tc.tile_pool(name="psum", bufs=2, space="PSUM"))
            p1pool = ctx.enter_context(tc.tile_pool(name="psum1", bufs=1, space="PSUM"))
            dpool = ctx.enter_context(tc.tile_pool(name="dram", bufs=2, space="DRAM"))
            d1pool = ctx.enter_context(tc.tile_pool(name="dram1", bufs=1, space="DRAM"))

            # ---- persistent SBUF tiles ----
            w0T = wpool.tile([P, 16, GC], bf16, tag="w0T")
            w1T = wpool.tile([P, 16, GC], bf16, tag="w1T")
            gbb = wpool.tile([1, 3 * GC], bf16, tag="gbb")
            m1t = wpool.tile([P, BG, 8, S], bf16, tag="m1t")
            c1t = wpool.tile([P, 2, S], fp32, tag="c1t")
            m2s = wpool.tile([P, BG, E], bf16, tag="m2s")
            w2hb = wpool.tile([P, 8, E], bf16, tag="w2hb")
            ident = wpool.tile([P, P], fp32, tag="ident")
            ones = wpool.tile([1, B], bf16, tag="ones")
            sc = wpool.tile([P, 8], fp32, tag="sc")

            h0f = [
                spool.tile([P, NCORES * B], bf16, tag=f"h0f{i}", name=f"h0f{i}")
                for i in range(2)
            ]
            h1f = [
                spool.tile([P, NCORES * B], bf16, tag=f"h1f{i}", name=f"h1f{i}")
                for i in range(2)
            ]
            of = [
                spool.tile([P, NCORES * 4 * BG], bf16, tag=f"of{i}", name=f"of{i}")
                for i in range(2)
            ]
            c0 = spool.tile([B, HC], fp32, tag="c0")
            c1 = spool.tile([B, HC], fp32, tag="c1")
            h1my = spool.tile([P, 8, BG], bf16, tag="h1my")

            # ---- prologue transients ----
            w1f = propool.tile([P, 8, H], fp32, tag="w1f")
            w2vf = propool.tile([P, 8, E], fp32, tag="w2vf")
            lw8 = propool.tile([P, 4, 2048], i8, tag="lw8")
            castbuf = propool.tile([P, 2048], fp32, tag="castbuf")
            st16 = propool.tile([P, H], i16, tag="st16")
            h16 = propool.tile([P, 8, S], i16, tag="h16")
            h16f = propool.tile([P, 8, S], fp32, tag="h16f")
            his16 = propool.tile([P, NCORES, B], i16, tag="his16")
            ois16 = propool.tile([P, NCORES * 4 * BG], i16, tag="ois16")

            # ---- basics ----
            nc.sync.dma_start(sc[:], d_sc[:])
            gbf = propool.tile([1, 3 * GC], fp32, tag="gbf")
            nc.sync.dma_start(
                gbf[:], d_gb[:].rearrange("l g -> (l g)").rearrange("(a x) -> a x", a=1)
            )
            nc.vector.tensor_copy(gbb[:], gbf[:])
            nc.vector.memset(ones[:], 1.0)
            masks.make_identity(nc, ident[:])
            nc.vector.memset(c0[:], 0.0)
            nc.vector.memset(c1[:], 0.0)

            pid = nc.vector.partition_id()

            # ---- blob AllGather ----
            agi = d1pool.tile([nb1], i16, tag="agi")
            ago = d1pool.tile([NCORES, nb1], i16, tag="ago")
            nc.sync.dma_start(agi[:], d_blob[:])
            nc.gpsimd.collective_compute(
                "AllGather",
                mybir.AluOpType.bypass,
                replica_groups=RG,
                ins=[agi.opt()],
                outs=[ago.opt()],
            )

            # ---- W1 -> w1f fp32 [p, vc, h] (int-valued) ----
            for vc in range(NCORES):
                nc.sync.dma_start(
                    st16[:],
                    ago[vc, OFF_W1 : OFF_W1 + N_W1].rearrange("(p h) -> p h", p=P),
                )
                nc.vector.tensor_copy(w1f[:, vc, :], st16[:])

            # ---- W2.T chunks: vc 0..7 -> w2vf fp32 int-valued;
            #      hc 0..7 -> w2hb bf16 real-valued (scale s_w2) ----
            for rc in range(16):
                k, half = rc // 2, rc % 2
                src = ago[
                    k, OFF_W2 + half * P * E : OFF_W2 + (half + 1) * P * E
                ].rearrange("(p e) -> p e", p=P)
                nc.scalar.dma_start(st16[:, 0:E], src)
                if rc < 8:
                    nc.vector.tensor_copy(w2vf[:, rc, :], st16[:, 0:E])
                else:
                    nc.scalar.activation(
                        w2hb[:, rc - 8, :], st16[:, 0:E], AF.Copy, scale=sc[:, 6:7]
                    )

            # ---- LSTM weights: int8 -> cast -> PE transpose -> scaled bf16 ----
            for l in range(L):
                wT = w0T if l == 0 else w1T
                nc.sync.dma_start(
                    lw8[:], d_lw[l].rearrange("(c p) k -> p c k", p=P)
                )
                for c in range(4):
                    nc.vector.tensor_copy(castbuf[:], lw8[:, c, :])
                    for kb in range(16):
                        ptw = ppool.tile([P, 2, GC], fp32, tag="pg", name=f"ptw{l}_{c}_{kb}")
                        nc.tensor.transpose(
                            ptw[:, 0, 0:P],
                            castbuf[:, kb * P : (kb + 1) * P],
                            ident[:],
                        )
                        nc.scalar.activation(
                            wT[:, kb, c * P : (c + 1) * P],
                            ptw[:, 0, 0:P],
                            AF.Copy,
                            scale=sc[:, 0:1],
                        )

            # ---- h_enc transposes + m1t/m2s compute (fp32, scales folded) ----
            for j in range(BG):
                for vc in range(8):
                    nc.sync.dma_start_transpose(
                        h16[:, vc, :], d_henc[j, :, vc * P : (vc + 1) * P]
                    )
                nc.vector.tensor_copy(h16f[:], h16[:])
                for kt in range(8):
                    pm = ppool.tile([P, 2, GC], fp32, tag="pg", name=f"pm{j}_{kt}")
                    for vc in range(8):
                        nc.tensor.matmul(
                            pm[:, 0, 0:S],
                            w1f[:, vc, kt * P : (kt + 1) * P],
                            h16f[:, vc, :],
                            start=(vc == 0),
                            stop=(vc == 7),
                        )
                    nc.scalar.activation(
                        m1t[:, j, kt, :], pm[:, 0, 0:S], AF.Copy, scale=sc[:, 4:5]
                    )
                pm2 = ppool.tile([P, 2, GC], fp32, tag="pg", name=f"pm2_{j}")
                for vc in range(8):
                    nc.tensor.matmul(
                        pm2[:, 0, :],
                        h16f[:, vc, :],
                        w2vf[:, vc, :],
                        start=(vc == 0),
                        stop=(vc == 7),
                    )
                nc.scalar.activation(
                    m2s[:, j, :], pm2[:, 0, :], AF.Copy, scale=sc[:, 5:6]
                )

            # ---- state inits from blob ----
            nc.sync.dma_start(
                his16[:], ago[:, off_h0 : off_h0 + N_HI].rearrange("k (p b) -> p k b", p=P)
            )
            nc.scalar.activation(
                h0f[1][:],
                his16[:].rearrange("p k b -> p (k b)"),
                AF.Copy,
                scale=sc[:, 2:3],
            )
            nc.sync.dma_start(
                his16[:], ago[:, off_h1 : off_h1 + N_HI].rearrange("k (p b) -> p k b", p=P)
            )
            nc.scalar.activation(
                h1f[1][:],
                his16[:].rearrange("p k b -> p (k b)"),
                AF.Copy,
                scale=sc[:, 2:3],
            )
            nc.sync.dma_start(
                ois16[:].rearrange("p (k c j) -> p k c j", k=NCORES, c=4),
                ago[:, off_oi : off_oi + N_OI].rearrange(
                    "k (p c j) -> p k c j", p=P, c=4
                ),
            )
            nc.scalar.activation(of[1][:], ois16[:], AF.Copy, scale=sc[:, 3:4])

            # ---- c1t rows ----
            nc.vector.memset(c1t[:], 0.0)
            for j in range(BG):
                nc.scalar.dma_start(
                    c1t[32 * (j % 4) : 32 * (j % 4) + 1, j // 4, :],
                    d_c1r[j].rearrange("(a s) -> a s", a=1),
                )

            def lstm_pointwise(g_sb, cst, h_out):
                """g_sb [B, 4*HC] gates i,f,g,o; updates cst, writes h_out [B,HC]."""
                gt = tpool.tile([B, HC], fp32, tag="pw_gt")
                ot = tpool.tile([B, HC], fp32, tag="pw_ot")
                ift = tpool.tile([B, 2 * HC], fp32, tag="pw_ift")
                nc.scalar.activation(ift[:], g_sb[:, 0 : 2 * HC], AF.Sigmoid)
                it, ft = ift[:, 0:HC], ift[:, HC : 2 * HC]
                nc.scalar.activation(gt[:], g_sb[:, 2 * HC : 3 * HC], AF.Tanh)
                nc.scalar.activation(ot[:], g_sb[:, 3 * HC : 4 * HC], AF.Sigmoid)
                t1 = tpool.tile([B, HC], fp32, tag="pw_t1")
                nc.vector.tensor_mul(t1[:], ft, cst[:])
                nc.vector.tensor_mul(gt[:], it, gt[:])
                nc.vector.tensor_add(cst[:], t1[:], gt[:])
                tc_ = tpool.tile([B, HC], fp32, tag="pw_tc")
                nc.scalar.activation(tc_[:], cst[:], AF.Tanh)
                nc.vector.tensor_mul(h_out[:], ot[:], tc_[:])

            def exchange(kind, src_sb, width, dst_tile):
                """Broadcast my [P,width] bf16 chunk into slot k of everyone's dst."""
                bi = dpool.tile([P, width], bf16, tag=f"agi{kind}", name=f"agi{kind}")
                bo = dpool.tile(
                    [P * NCORES, width], bf16, tag=f"ago{kind}", name=f"ago{kind}"
                )
                nc.gpsimd.dma_start(bi[:], src_sb)
                nc.gpsimd.collective_compute(
                    "AllGather",
                    mybir.AluOpType.bypass,
                    replica_groups=RG,
                    ins=[bi.opt()],
                    outs=[bo.opt()],
                )
                nc.gpsimd.dma_start(
                    dst_tile[:].rearrange("p (k w) -> p k w", k=NCORES),
                    bo[:].rearrange("(k p) w -> p k w", p=P),
                )

            x_step = P * 4 * B

            for t in range(n_steps):
                # ---- x load (int16 from AG'd blob) + dequant to bf16 ----
                xi16 = xpool.tile([P, 4, B], i16, tag="xi16")
                kc, tt = t // (n_pad // NCORES), t % (n_pad // NCORES)
                nc.scalar.dma_start(
                    xi16[:],
                    ago[kc, OFF_X + tt * x_step : OFF_X + (tt + 1) * x_step].rearrange(
                        "(p c b) -> p c b", p=P, c=4
                    ),
                )
                xt = xpool.tile([P, 4, B], bf16, tag="xt")
                nc.scalar.activation(xt[:], xi16[:], AF.Copy, scale=sc[:, 1:2])

                h0f_r = h0f[(t - 1) % 2]
                h1f_r = h1f[(t - 1) % 2]
                of_r = of[(t - 1) % 2]
                of_rv = of_r[:].rearrange("p (k c j) -> p c k j", k=NCORES, c=4)
                o4 = tpool.tile([P, 4, B], bf16, tag="o4")
                nc.vector.tensor_copy(
                    o4[:].rearrange("p c (k j) -> p c k j", k=NCORES), of_rv
                )

                # ---- gates0: bias + K = [x(4) | o(4) | h0(8)] ----
                pg0 = ppool.tile([P, 2, GC], fp32, tag="pg")
                order0 = [0, 1, 2, 3] + [8, 9, 10, 11, 12, 13, 14, 15] + [4, 5, 6, 7]
                nc.tensor.matmul(
                    pg0[0:B, 0, :], ones[:], gbb[:, 0:GC],
                    start=True, stop=False, tile_position=(0, 0),
                )
                for i, kt in enumerate(order0):
                    if kt < 4:
                        lhsT = xt[:, kt, :]
                    elif kt < 8:
                        lhsT = o4[:, kt - 4, :]
                    else:
                        lhsT = h0f_r[:, (kt - 8) * B : (kt - 7) * B]
                    hf = (i + 1) % 2
                    nc.tensor.matmul(
                        pg0[64 * hf : 64 * hf + 64, hf, :],
                        lhsT,
                        w0T[:, kt, :],
                        start=(i < 1),
                        stop=(i >= 14),
                        tile_position=(0, 64 * hf),
                    )
                g0 = tpool.tile([B, GC], fp32, tag="g0")
                nc.scalar.activation(g0[:], pg0[0:64, 0, :], AF.Copy)
                nc.vector.tensor_add(g0[:], g0[:], pg0[64:128, 1, :])
                h0m = tpool.tile([B, HC], fp32, tag="h0m")
                lstm_pointwise(g0, c0, h0m)

                # ---- transpose h0m -> [HC, B] bf16, AG -> h0f ----
                pt0 = p1pool.tile([P, P], fp32, tag="ptr", name="pt0")
                nc.tensor.transpose(pt0[:, 0:B], h0m[:], ident[0:B, 0:B])
                h0T = tpool.tile([P, B], bf16, tag="h0T")
                nc.vector.tensor_copy(h0T[:], pt0[:, 0:B])
                exchange(0, h0T[:], B, h0f[t % 2])

                # ---- gates1: bias + K = [h0(8) | h1(8)] ----
                h0f_w = h0f[t % 2]
                pg1 = ppool.tile([P, 2, GC], fp32, tag="pg")
                order1 = [8, 9, 10, 11, 12, 13, 14, 15] + [0, 1, 2, 3, 4, 5, 6, 7]
                nc.tensor.matmul(
                    pg1[0:B, 0, :], ones[:], gbb[:, GC : 2 * GC],
                    start=True, stop=False, tile_position=(0, 0),
                )
                for i, kt in enumerate(order1):
                    lhsT = (
                        h0f_w[:, kt * B : (kt + 1) * B]
                        if kt < 8
                        else h1f_r[:, (kt - 8) * B : (kt - 7) * B]
                    )
                    hf = (i + 1) % 2
                    nc.tensor.matmul(
                        pg1[64 * hf : 64 * hf + 64, hf, :],
                        lhsT,
                        w1T[:, kt, :],
                        start=(i < 1),
                        stop=(i >= 14),
                        tile_position=(0, 64 * hf),
                    )
                g1 = tpool.tile([B, GC], fp32, tag="g1")
                nc.scalar.activation(g1[:], pg1[0:64, 0, :], AF.Copy)
                nc.vector.tensor_add(g1[:], g1[:], pg1[64:128, 1, :])
                h1m = tpool.tile([B, HC], fp32, tag="h1m")
                lstm_pointwise(g1, c1, h1m)

                # ---- transpose h1m, AG -> h1f ----
                pt1 = p1pool.tile([P, P], fp32, tag="ptr", name="pt1")
                nc.tensor.transpose(pt1[:, 0:B], h1m[:], ident[0:B, 0:B])
                h1T = tpool.tile([P, B], bf16, tag="h1T")
                nc.vector.tensor_copy(h1T[:], pt1[:, 0:B])
                exchange(1, h1T[:], B, h1f[t % 2])

                # ---- select my batch columns of h1 (query) ----
                h1f_wv = h1f[t % 2][:].rearrange("p (kc b) -> p kc b", kc=8)
                nc.vector.tensor_copy(h1my[:], h1f_wv[:, :, bass.ts(pid, BG)])

                # ---- scores: per-b matvec via tile_position packing ----
                psc = p1pool.tile([P, 2, S], fp32, tag="psc")
                nc.vector.memset(psc[:], 0.0)
                for j in range(BG):
                    half, row = j // 4, 32 * (j % 4)
                    for kt in range(8):
                        nc.tensor.matmul(
                            psc[row : row + 1, half, :],
                            h1my[:, kt, j : j + 1],
                            m1t[:, j, kt, :],
                            start=(kt == 0),
                            stop=(kt == 7),
                            tile_position=(0, row),
                        )
                # ---- softmax over the two halves (garbage rows are fine) ----
                a_sb = tpool.tile([P, 2, S], fp32, tag="a_sb")
                stat = tpool.tile([P, 4], fp32, tag="stat")
                for half in range(2):
                    nc.vector.tensor_add(
                        a_sb[:, half, :], psc[:, half, :], c1t[:, half, :]
                    )
                    nm = stat[:, 2 * half : 2 * half + 1]
                    nc.vector.tensor_reduce(
                        nm, a_sb[:, half, :], axis=AX.X, op=mybir.AluOpType.max,
                        negate=True,
                    )
                    sm = stat[:, 2 * half + 1 : 2 * half + 2]
                    nc.scalar.activation(
                        a_sb[:, half, :], a_sb[:, half, :], AF.Exp, bias=nm,
                        accum_out=sm,
                    )
                    nc.vector.reciprocal(sm, sm)
                    nc.vector.tensor_scalar_mul(a_sb[:, half, :], a_sb[:, half, :], sm)

                # ---- transpose a -> columns; build block-diag lhsT (bf16) ----
                paT = p1pool.tile([P, 2, S], fp32, tag="psc", name="paT")
                nc.tensor.transpose(paT[:, 0, :], a_sb[:, 0, :], ident[:])
                nc.tensor.transpose(paT[:, 1, :], a_sb[:, 1, :], ident[:])
                abd = tpool.tile([P, BG * BG], bf16, tag="abd")
                nc.vector.memset(abd[:], 0.0)
                nc.vector.tensor_copy(
                    abd[:, 0 : BG * BG : 9].rearrange("p (a b) -> p a b", a=2),
                    paT[:].rearrange("p h (c x) -> p h c x", c=4)[:, :, :, 0:1],
                )

                # ---- z = b2 + blockdiag(a) @ M2stack + h1my.T @ W2h.T ----
                pz = p1pool.tile([BG, E], fp32, tag="pz")
                nc.tensor.matmul(
                    pz[:], ones[:, 0:BG], gbb[:, 2 * GC :], start=True, stop=False
                )
                for j in range(BG):
                    nc.tensor.matmul(
                        pz[:],
                        abd[:, j * BG : (j + 1) * BG],
                        m2s[:, j, :],
                        start=False,
                        stop=False,
                    )
                for kt in range(8):
                    nc.tensor.matmul(
                        pz[:], h1my[:, kt, :], w2hb[:, kt, :], start=False,
                        stop=(kt == 7),
                    )
                o_sb = tpool.tile([BG, E], fp32, tag="o_sb")
                nc.scalar.activation(o_sb[:], pz[:], AF.Tanh)

                # ---- write output (int16) ----
                oq = tpool.tile([BG, E], i16, tag="oq")
                nc.scalar.activation(oq[:], o_sb[:], AF.Copy, scale=O_SCALE)
                nc.scalar.dma_start(d_out[t], oq[:])

                # ---- transpose o chunks -> [P, 4, BG] bf16, AG -> of ----
                poT = p1pool.tile([P, 4, BG], fp32, tag="ptr", name="poT")
                for cchunk in range(4):
                    nc.tensor.transpose(
                        poT[:, cchunk, :],
                        o_sb[:, cchunk * P : (cchunk + 1) * P],
                        ident[0:BG, 0:BG],
                    )
                oT = tpool.tile([P, 4 * BG], bf16, tag="oT")
                nc.vector.tensor_copy(
                    oT[:].rearrange("p (c j) -> p c j", c=4), poT[:]
                )
                exchange(2, oT[:], 4 * BG, of[t % 2])

    nc.compile()
    return nc


def _quant(x, bits):
    m = float((1 << (bits - 1)) - 1)
    s = float(np.abs(x).max())
    s = s / m if s > 0 else 1.0
    return np.rint(x * (1.0 / s)), np.float32(s)


def _host_prep(inputs: dict, n_steps: int):
    """Build per-core in_maps (quantize + contiguous reshapes only)."""
    f32, i16, i8 = np.float32, np.int16, np.int8
    n_pad = ((n_steps + NCORES - 1) // NCORES) * NCORES
    n_x, off_h0, off_h1, off_oi, nb1 = _blob_layout(n_pad)

    tgt = np.asarray(inputs["tgt_batch"])
    h_enc = np.asarray(inputs["h_encoder"], f32)
    emb = np.asarray(inputs["emb"], f32)
    out_init = np.asarray(inputs["output_init"], f32)
    hid_init = np.asarray(inputs["hidden_init"], f32)
    W_ih = np.asarray(inputs["W_ih"], f32)
    W_hh = np.asarray(inputs["W_hh"], f32)
    b_ih = np.asarray(inputs["b_ih"], f32)
    b_hh = np.asarray(inputs["b_hh"], f32)
    W1 = np.asarray(inputs["W1"], f32)
    b1v = np.asarray(inputs["b1"], f32)
    W2 = np.asarray(inputs["W2"], f32)
    b2v = np.asarray(inputs["b2"], f32)

    # LSTM weights: int8, gate-dim sharded, natural [rows, k_in] layout
    Wcat = np.concatenate([W_ih, W_hh], axis=2)  # [2, 4096, 2048]
    wq, s_w = _quant(Wcat, 8)
    lw = np.ascontiguousarray(
        wq.astype(i8).reshape(2, 4, 8, P, 2048).transpose(2, 0, 1, 3, 4)
    ).reshape(NCORES, L, GC, 2048)

    # h_encoder: int16, batch-sharded, natural [S, V] layout
    hq, s_h = _quant(h_enc, 16)
    g_henc = hq.astype(i16).reshape(NCORES, BG, S, V)

    # x sequence: int16, feature-major [T, P, 4, B], T-sharded
    xs = emb[tgt[:n_steps]]  # [n, B, E]
    xq, s_x = _quant(xs, 16)
    xq = xq.astype(i16)
    if n_pad != n_steps:
        xq = np.concatenate([xq, np.zeros((n_pad - n_steps, B, E), i16)], axis=0)
    xfold = np.ascontiguousarray(
        xq.transpose(0, 2, 1).reshape(n_pad, 4, P, B).transpose(0, 2, 1, 3)
    )  # [n_pad, P, 4, B]

    # W1 / W2.T: int16, row-sharded
    w1q, s_w1 = _quant(W1, 16)
    w1s = w1q.astype(i16).reshape(NCORES, P, H)
    w2q, s_w2 = _quant(W2, 16)
    w2s = np.ascontiguousarray(w2q.astype(i16).T).reshape(NCORES, 2 * P, E)

    # state inits: int16, feature-chunk sharded
    hi_s = float(np.abs(hid_init).max())
    s_hi = hi_s / 32767.0 if hi_s > 0 else 1.0
    h0s = np.rint(hid_init[0].T * (1.0 / s_hi)).astype(i16).reshape(NCORES, P, B)
    h1s = np.rint(hid_init[1].T * (1.0 / s_hi)).astype(i16).reshape(NCORES, P, B)
    oi_s = float(np.abs(out_init).max())
    s_oi = oi_s / 32767.0 if oi_s > 0 else 1.0
    oi4 = np.rint(out_init.T * (1.0 / s_oi)).astype(i16)
    ois = np.ascontiguousarray(
        oi4.reshape(4, P, NCORES, BG).transpose(2, 1, 0, 3)
    )  # [k, P, 4, BG]

    # blob assembly
    blob = np.empty((NCORES, nb1), i16)
    blob[:, OFF_W1 : OFF_W1 + N_W1] = w1s.reshape(NCORES, -1)
    blob[:, OFF_W2 : OFF_W2 + N_W2] = w2s.reshape(NCORES, -1)
    blob[:, OFF_X : OFF_X + n_x] = xfold.reshape(NCORES, -1)
    blob[:, off_h0 : off_h0 + N_HI] = h0s.reshape(NCORES, -1)
    blob[:, off_h1 : off_h1 + N_HI] = h1s.reshape(NCORES, -1)
    blob[:, off_oi : off_oi + N_OI] = ois.reshape(NCORES, -1)

    # c1 rows (host fp32, exact): c1[b] = h_enc[b] @ b1
    c1v = (h_enc.reshape(-1, V) @ b1v).reshape(B, S).astype(f32)
    g_c1r = np.ascontiguousarray(c1v.reshape(NCORES, BG, S))

    # gate biases + b2
    b01 = (b_ih + b_hh).reshape(2, 4, NCORES, P).transpose(2, 0, 1, 3).reshape(
        NCORES, 2, GC
    )
    g_gb = np.empty((NCORES, 3, GC), f32)
    g_gb[:, 0:2] = b01
    g_gb[:, 2] = b2v
    g_gb = np.ascontiguousarray(g_gb)

    # scales [P, 8]: s_w, s_x, s_hi, s_oi, s_h*s_w1, s_h*s_w2, s_w2, 0
    srow = np.array(
        [s_w, s_x, s_hi, s_oi, s_h * s_w1, s_h * s_w2, s_w2, 0.0], f32
    )
    g_sc = np.broadcast_to(srow, (P, 8)).copy()

    in_maps = []
    for k in range(NCORES):
        in_maps.append(
            {
                "blob": blob[k],
                "lw": lw[k],
                "henc": g_henc[k],
                "c1r": g_c1r[k],
                "gb": g_gb[k],
                "sc": g_sc,
            }
        )
    return in_maps


def _assemble(outs, n_steps):
    """outs: list of per-core [n, BG, E] int16 -> [B, n, E] fp32."""
    full = np.stack(outs, axis=0)  # [k, n, BG, E]
    full = full.transpose(0, 2, 1, 3).reshape(B, n_steps, E)
    return full.astype(np.float32) * np.float32(1.0 / O_SCALE)


def _get_exec(n_steps: int):
    if n_steps in _CACHE:
        return _CACHE[n_steps]
    import jax
    import jax.numpy as jnp
    from jax.sharding import NamedSharding
    from concourse import bass2jax
    import concourse.mybir as mybir

    nc = _build(n_steps)
    bass2jax.install_neuronx_cc_hook()

    partition_name = nc.partition_id_tensor.name if nc.partition_id_tensor else None
    in_names, out_names, out_avals = [], [], []
    for alloc in nc.m.functions[0].allocations:
        if not isinstance(alloc, mybir.MemoryLocationSet):
            continue
        name = alloc.memorylocations[0].name
        if alloc.kind == "ExternalInput":
            if name != partition_name:
                in_names.append(name)
        elif alloc.kind == "ExternalOutput":
            out_names.append(name)
            out_avals.append(
                jax.core.ShapedArray(
                    tuple(alloc.tensor_shape), mybir.dt.np(alloc.dtype)
                )
            )
    n_params = len(in_names)
    all_names = list(in_names) + list(out_names)
    if partition_name is not None:
        all_names.append(partition_name)

    def _body(*args):
        operands = list(args)
        if partition_name is not None:
            operands.append(bass2jax.partition_id_tensor())
        outs = bass2jax._bass_exec_p.bind(
            *operands,
            out_avals=tuple(out_avals),
            in_names=tuple(all_names),
            out_names=tuple(out_names),
            lowering_input_output_aliases=(),
            sim_require_finite=True,
            sim_require_nnan=True,
            nc=nc,
        )
        return tuple(outs)

    devices = jax.devices()[:NCORES]
    mesh = bass2jax.Mesh(np.asarray(devices), ("core",))
    PS = bass2jax.PartitionSpec
    in_specs = (PS("core"),) * (n_params + len(out_names))
    out_specs = (PS("core"),) * len(out_names)
    donate = tuple(range(n_params, n_params + len(out_names)))
    sharded = jax.jit(
        bass2jax.shard_map(
            _body, mesh=mesh, in_specs=in_specs, out_specs=out_specs, check_rep=False
        ),
        donate_argnums=donate,
        keep_unused=True,
    )
    shardings = tuple(NamedSharding(mesh, PS("core")) for _ in out_avals)
    gshapes = [(NCORES * a.shape[0], *a.shape[1:]) for a in out_avals]
    gdtypes = [a.dtype for a in out_avals]
    zfn = jax.jit(
        lambda: tuple(jnp.zeros(s, d) for s, d in zip(gshapes, gdtypes)),
        out_shardings=shardings,
    )
    state = {
        "sharded": sharded,
        "zfn": zfn,
        "in_names": in_names,
        "out_names": out_names,
        "out_avals": out_avals,
        "nc": nc,
    }
    _CACHE[n_steps] = state
    return state


def run(inputs: dict, n_steps: int = T):
    st = _get_exec(n_steps)
    in_maps = _host_prep(inputs, n_steps)
    gargs = [
        np.concatenate([in_maps[k][name] for k in range(NCORES)], axis=0).reshape(
            NCORES * np.asarray(in_maps[0][name]).shape[0],
            *np.asarray(in_maps[0][name]).shape[1:],
        )
        for name in st["in_names"]
    ]
    zeros = st["zfn"]()
    outs = st["sharded"](*gargs, *zeros)
    oname_i = st["out_names"].index("out")
    o = np.asarray(outs[oname_i])  # [8*n, BG, E] int16
    o = o.reshape(NCORES, n_steps, BG, E)
    return _assemble(list(o), n_steps)


def kernel(**inputs) -> np.ndarray:
    return run(inputs, T)


# revision 17
# speedup vs baseline: 8.3076x; 1.0716x over previous
"""AttentionDecoder Trainium2 kernel: 8-way model-parallel LSTM+attention decoder.

v2 — optimized for end-to-end wall clock through the axon tunnel (~82MB/s):
  - Quantized transfer: LSTM weights int8, h_encoder/W1/W2/xseq int16,
    output int16 (~43MB/call vs 210MB fp32 baseline).
  - Attention precompute (M1T = (h_enc @ W1).T, M2 = h_enc @ W2v.T) moved
    from host numpy (was 4.3s) onto the device prologue (fp32 PE matmuls
    on exact int16 operands, scales folded into the PSUM->SBUF copies).
  - Shared tensors (W1, W2, xseq, state inits) shipped as 1/8 shards and
    AllGathered on device.
  - All layout transposes on device (PE transpose / DMA XBAR transpose);
    host prep is quantize + contiguous reshapes only.
  - Steady-state exec path caches the jitted shard_map wrapper (no per-call
    retrace) and creates donated output zeros on device.
  - Step loop matmuls in bf16 (weights already <= 8-bit precision),
    pointwise/softmax in fp32, per-step AllGathers in bf16.

Numerics validated vs reference: rel err ~2.8e-3 (tolerance 2e-2).

Layout summary (per core k):
  - Weights sharded over the 4H gate dim: core k owns gate rows
    [g*H + k*128, g*H + (k+1)*128) for g in 0..3 of both layers.
  - Activations feature-major [feat, batch]; gates matmuls are
    lhsT = activation chunk [K=128 feats, M=B], rhs = weight.T chunk.
  - Per timestep: 3 bf16 AllGathers (h0, h1, o) across the 8 cores.
  - Attention per-core batch shard: core k owns batches 8k..8k+7.
"""

import warnings

warnings.filterwarnings("ignore")

import numpy as np

VOCAB, E, H, L, B, T, S, V = 32000, 512, 1024, 2, 64, 64, 128, 1024
NCORES = 8
P = 128
BG = B // NCORES  # 8 batches per core for attention
HC = H // NCORES  # 128 hidden feats per core
GC = 4 * HC  # 512 gate rows per core

O_SCALE = 32000.0  # fixed output quant scale (tanh output, |o| <= 1)

# ---- AllGather blob layout (int16 elems, per-core contribution) ----
N_W1 = P * H  # [128, 1024] W1 row chunk
N_W2 = 2 * P * E  # [256, 512] W2.T row chunk
N_HI = P * B  # [128, 64] hidden-init feature chunk
N_OI = P * 4 * BG  # [128, 4, 8] output-init chunk
OFF_W1 = 0
OFF_W2 = OFF_W1 + N_W1
OFF_X = OFF_W2 + N_W2


def _blob_layout(n_pad):
    n_x = (n_pad // NCORES) * P * 4 * B
    off_h0 = OFF_X + n_x
    off_h1 = off_h0 + N_HI
    off_oi = off_h1 + N_HI
    nb1 = off_oi + N_OI
    return n_x, off_h0, off_h1, off_oi, nb1


_CACHE = {}


def _build(n_steps: int):
    import concourse.bass as bass
    import concourse.bacc as bacc
    import concourse.mybir as mybir
    import concourse.tile as tile
    from concourse import masks

    fp32 = mybir.dt.float32
    bf16 = mybir.dt.bfloat16
    i16 = mybir.dt.int16
    i8 = mybir.dt.int8
    AF = mybir.ActivationFunctionType
    AX = mybir.AxisListType

    n_pad = ((n_steps + NCORES - 1) // NCORES) * NCORES
    n_x, off_h0, off_h1, off_oi, nb1 = _blob_layout(n_pad)
    x_per_core = (n_pad // NCORES) * P * 4 * B

    nc = bacc.Bacc("TRN2", target_bir_lowering=False, debug=False, num_devices=NCORES)

    # ---- DRAM I/O ----
    d_blob = nc.dram_tensor("blob", [nb1], i16, kind="ExternalInput")
    d_lw = nc.dram_tensor("lw", [L, GC, 2048], i8, kind="ExternalInput")
    d_henc = nc.dram_tensor("henc", [BG, S, V], i16, kind="ExternalInput")
    d_c1r = nc.dram_tensor("c1r", [BG, S], fp32, kind="ExternalInput")
    d_gb = nc.dram_tensor("gb", [3, GC], fp32, kind="ExternalInput")
    d_sc = nc.dram_tensor("sc", [P, 8], fp32, kind="ExternalInput")
    d_out = nc.dram_tensor("out", [n_steps, BG, E], i16, kind="ExternalOutput")

    RG = [list(range(NCORES))]

    with tile.TileContext(nc) as tc:
        import contextlib

        ctx = contextlib.ExitStack()
        with ctx:
            wpool = ctx.enter_context(tc.tile_pool(name="weights", bufs=1))
            spool = ctx.enter_context(tc.tile_pool(name="state", bufs=1))
            propool = ctx.enter_context(tc.tile_pool(name="pro", bufs=1))
            xpool = ctx.enter_context(tc.tile_pool(name="x", bufs=2))
            tpool = ctx.enter_context(tc.tile_pool(name="tmp", bufs=2))
            ppool = ctx.enter_context(tc.tile_pool(name="psum", bufs=2, space="PSUM"))
            p1pool = ctx.enter_context(tc.tile_pool(name="psum1", bufs=1, space="PSUM"))
            dpool = ctx.enter_context(tc.tile_pool(name="dram", bufs=2, space="DRAM"))
            d1pool = ctx.enter_context(tc.tile_pool(name="dram1", bufs=1, space="DRAM"))

            # ---- persistent SBUF tiles ----
            w0T = wpool.tile([P, 16, GC], bf16, tag="w0T")
            w1T = wpool.tile([P, 16, GC], bf16, tag="w1T")
            gbb = wpool.tile([1, 3 * GC], bf16, tag="gbb")
            m1t = wpool.tile([P, BG, 8, S], bf16, tag="m1t")
            c1t = wpool.tile([P, 2, S], fp32, tag="c1t")
            m2s = wpool.tile([P, BG, E], bf16, tag="m2s")
            w2hb = wpool.tile([P, 8, E], bf16, tag="w2hb")
            ident = wpool.tile([P, P], fp32, tag="ident")
            ones = wpool.tile([1, B], bf16, tag="ones")
            sc = wpool.tile([P, 8], fp32, tag="sc")

            h0f = [
                spool.tile([P, NCORES * B], bf16, tag=f"h0f{i}", name=f"h0f{i}")
                for i in range(2)
            ]
            h1f = [
                spool.tile([P, NCORES * B], bf16, tag=f"h1f{i}", name=f"h1f{i}")
                for i in range(2)
            ]
            of = [
                spool.tile([P, NCORES * 4 * BG], bf16, tag=f"of{i}", name=f"of{i}")
                for i in range(2)
            ]
            c0 = spool.tile([B, HC], fp32, tag="c0")
            c1 = spool.tile([B, HC], fp32, tag="c1")
            h1my = spool.tile([P, 8, BG], bf16, tag="h1my")

            # ---- prologue transients ----
            w1f = propool.tile([P, 8, H], fp32, tag="w1f")
            w2vf = propool.tile([P, 8, E], fp32, tag="w2vf")
            lw8 = propool.tile([P, 4, 2048], i8, tag="lw8")
            castbuf = propool.tile([P, 2048], fp32, tag="castbuf")
            st16 = propool.tile([P, H], i16, tag="st16")
            h16 = propool.tile([P, 8, S], i16, tag="h16")
            h16f = propool.tile([P, 8, S], fp32, tag="h16f")
            his16 = propool.tile([P, NCORES, B], i16, tag="his16")
            ois16 = propool.tile([P, NCORES * 4 * BG], i16, tag="ois16")

            # ---- basics ----
            nc.sync.dma_start(sc[:], d_sc[:])
            gbf = propool.tile([1, 3 * GC], fp32, tag="gbf")
            nc.sync.dma_start(
                gbf[:], d_gb[:].rearrange("l g -> (l g)").rearrange("(a x) -> a x", a=1)
            )
            nc.vector.tensor_copy(gbb[:], gbf[:])
            nc.vector.memset(ones[:], 1.0)
            masks.make_identity(nc, ident[:])
            nc.vector.memset(c0[:], 0.0)
            nc.vector.memset(c1[:], 0.0)

            pid = nc.vector.partition_id()

            # ---- blob AllGather ----
            agi = d1pool.tile([nb1], i16, tag="agi")
            ago = d1pool.tile([NCORES, nb1], i16, tag="ago")
            nc.sync.dma_start(agi[:], d_blob[:])
            nc.gpsimd.collective_compute(
                "AllGather",
                mybir.AluOpType.bypass,
                replica_groups=RG,
                ins=[agi.opt()],
                outs=[ago.opt()],
            )

            # ---- W1 -> w1f fp32 [p, vc, h] (int-valued) ----
            for vc in range(NCORES):
                nc.sync.dma_start(
                    st16[:],
                    ago[vc, OFF_W1 : OFF_W1 + N_W1].rearrange("(p h) -> p h", p=P),
                )
                nc.vector.tensor_copy(w1f[:, vc, :], st16[:])

            # ---- W2.T chunks: vc 0..7 -> w2vf fp32 int-valued;
            #      hc 0..7 -> w2hb bf16 real-valued (scale s_w2) ----
            for rc in range(16):
                k, half = rc // 2, rc % 2
                src = ago[
                    k, OFF_W2 + half * P * E : OFF_W2 + (half + 1) * P * E
                ].rearrange("(p e) -> p e", p=P)
                nc.scalar.dma_start(st16[:, 0:E], src)
                if rc < 8:
                    nc.vector.tensor_copy(w2vf[:, rc, :], st16[:, 0:E])
                else:
                    nc.scalar.activation(
                        w2hb[:, rc - 8, :], st16[:, 0:E], AF.Copy, scale=sc[:, 6:7]
                    )

            # ---- LSTM weights: int8 -> cast -> PE transpose -> scaled bf16 ----
            for l in range(L):
                wT = w0T if l == 0 else w1T
                nc.sync.dma_start(
                    lw8[:], d_lw[l].rearrange("(c p) k -> p c k", p=P)
                )
                for c in range(4):
                    nc.vector.tensor_copy(castbuf[:], lw8[:, c, :])
                    for kb in range(16):
                        ptw = ppool.tile([P, 2, GC], fp32, tag="pg", name=f"ptw{l}_{c}_{kb}")
                        nc.tensor.transpose(
                            ptw[:, 0, 0:P],
                            castbuf[:, kb * P : (kb + 1) * P],
                            ident[:],
                        )
                        nc.scalar.activation(
                            wT[:, kb, c * P : (c + 1) * P],
                            ptw[:, 0, 0:P],
                            AF.Copy,
                            scale=sc[:, 0:1],
                        )

            # ---- h_enc transposes + m1t/m2s compute (fp32, scales folded) ----
            for j in range(BG):
                for vc in range(8):
                    nc.sync.dma_start_transpose(
                        h16[:, vc, :], d_henc[j, :, vc * P : (vc + 1) * P]
                    )
                nc.vector.tensor_copy(h16f[:], h16[:])
                for kt in range(8):
                    pm = ppool.tile([P, 2, GC], fp32, tag="pg", name=f"pm{j}_{kt}")
                    for vc in range(8):
                        nc.tensor.matmul(
                            pm[:, 0, 0:S],
                            w1f[:, vc, kt * P : (kt + 1) * P],
                            h16f[:, vc, :],
                            start=(vc == 0),
                            stop=(vc == 7),
                        )
                    nc.scalar.activation(
                        m1t[:, j, kt, :], pm[:, 0, 0:S], AF.Copy, scale=sc[:, 4:5]
                    )
                pm2 = ppool.tile([P, 2, GC], fp32, tag="pg", name=f"pm2_{j}")
                for vc in range(8):
                    nc.tensor.matmul(
                        pm2[:, 0, :],
                        h16f[:, vc, :],
                        w2vf[:, vc, :],
                        start=(vc == 0),
                        stop=(vc == 7),
                    )
                nc.scalar.activation(
                    m2s[:, j, :], pm2[:, 0, :], AF.Copy, scale=sc[:, 5:6]
                )

            # ---- state inits from blob ----
            nc.sync.dma_start(
                his16[:], ago[:, off_h0 : off_h0 + N_HI].rearrange("k (p b) -> p k b", p=P)
            )
            nc.scalar.activation(
                h0f[1][:],
                his16[:].rearrange("p k b -> p (k b)"),
                AF.Copy,
                scale=sc[:, 2:3],
            )
            nc.sync.dma_start(
                his16[:], ago[:, off_h1 : off_h1 + N_HI].rearrange("k (p b) -> p k b", p=P)
            )
            nc.scalar.activation(
                h1f[1][:],
                his16[:].rearrange("p k b -> p (k b)"),
                AF.Copy,
                scale=sc[:, 2:3],
            )
            nc.sync.dma_start(
                ois16[:].rearrange("p (k c j) -> p k c j", k=NCORES, c=4),
                ago[:, off_oi : off_oi + N_OI].rearrange(
                    "k (p c j) -> p k c j", p=P, c=4
                ),
            )
            nc.scalar.activation(of[1][:], ois16[:], AF.Copy, scale=sc[:, 3:4])

            # ---- c1t rows ----
            nc.vector.memset(c1t[:], 0.0)
            for j in range(BG):
                nc.scalar.dma_start(
                    c1t[32 * (j % 4) : 32 * (j % 4) + 1, j // 4, :],
                    d_c1r[j].rearrange("(a s) -> a s", a=1),
                )

            def lstm_pointwise(g_sb, cst, h_out):
                """g_sb [B, 4*HC] gates i,f,g,o; updates cst, writes h_out [B,HC]."""
                gt = tpool.tile([B, HC], fp32, tag="pw_gt")
                ot = tpool.tile([B, HC], fp32, tag="pw_ot")
                ift = tpool.tile([B, 2 * HC], fp32, tag="pw_ift")
                nc.scalar.activation(ift[:], g_sb[:, 0 : 2 * HC], AF.Sigmoid)
                it, ft = ift[:, 0:HC], ift[:, HC : 2 * HC]
                nc.scalar.activation(gt[:], g_sb[:, 2 * HC : 3 * HC], AF.Tanh)
                nc.scalar.activation(ot[:], g_sb[:, 3 * HC : 4 * HC], AF.Sigmoid)
                t1 = tpool.tile([B, HC], fp32, tag="pw_t1")
                nc.vector.tensor_mul(t1[:], ft, cst[:])
                nc.vector.tensor_mul(gt[:], it, gt[:])
                nc.vector.tensor_add(cst[:], t1[:], gt[:])
                tc_ = tpool.tile([B, HC], fp32, tag="pw_tc")
                nc.scalar.activation(tc_[:], cst[:], AF.Tanh)
                nc.vector.tensor_mul(h_out[:], ot[:], tc_[:])

            def exchange(kind, src_sb, width, dst_tile):
                """Broadcast my [P,width] bf16 chunk into slot k of everyone's dst."""
                bi = dpool.tile([P, width], bf16, tag=f"agi{kind}", name=f"agi{kind}")
                bo = dpool.tile(
                    [P * NCORES, width], bf16, tag=f"ago{kind}", name=f"ago{kind}"
                )
                nc.gpsimd.dma_start(bi[:], src_sb)
                nc.gpsimd.collective_compute(
                    "AllGather",
                    mybir.AluOpType.bypass,
                    replica_groups=RG,
                    ins=[bi.opt()],
                    outs=[bo.opt()],
                )
                nc.gpsimd.dma_start(
                    dst_tile[:].rearrange("p (k w) -> p k w", k=NCORES),
                    bo[:].rearrange("(k p) w -> p k w", p=P),
                )

            x_step = P * 4 * B

            for t in range(n_steps):
                # ---- x load (int16 from AG'd blob) + dequant to bf16 ----
                xi16 = xpool.tile([P, 4, B], i16, tag="xi16")
                kc, tt = t // (n_pad // NCORES), t % (n_pad // NCORES)
                nc.scalar.dma_start(
                    xi16[:],
                    ago[kc, OFF_X + tt * x_step : OFF_X + (tt + 1) * x_step].rearrange(
                        "(p c b) -> p c b", p=P, c=4
                    ),
                )
                xt = xpool.tile([P, 4, B], bf16, tag="xt")
                nc.scalar.activation(xt[:], xi16[:], AF.Copy, scale=sc[:, 1:2])

                h0f_r = h0f[(t - 1) % 2]
                h1f_r = h1f[(t - 1) % 2]
                of_r = of[(t - 1) % 2]
                of_rv = of_r[:].rearrange("p (k c j) -> p c k j", k=NCORES, c=4)
                o4 = tpool.tile([P, 4, B], bf16, tag="o4")
                nc.vector.tensor_copy(
                    o4[:].rearrange("p c (k j) -> p c k j", k=NCORES), of_rv
                )

                # ---- gates0: bias + K = [x(4) | o(4) | h0(8)] ----
                pg0 = ppool.tile([P, 2, GC], fp32, tag="pg")
                order0 = [0, 1, 2, 3] + [8, 9, 10, 11, 12, 13, 14, 15] + [4, 5, 6, 7]
                nc.tensor.matmul(
                    pg0[0:B, 0, :], ones[:], gbb[:, 0:GC],
                    start=True, stop=False, tile_position=(0, 0),
                )
                for i, kt in enumerate(order0):
                    if kt < 4:
                        lhsT = xt[:, kt, :]
                    elif kt < 8:
                        lhsT = o4[:, kt - 4, :]
                    else:
                        lhsT = h0f_r[:, (kt - 8) * B : (kt - 7) * B]
                    hf = (i + 1) % 2
                    nc.tensor.matmul(
                        pg0[64 * hf : 64 * hf + 64, hf, :],
                        lhsT,
                        w0T[:, kt, :],
                        start=(i < 1),
                        stop=(i >= 14),
                        tile_position=(0, 64 * hf),
                    )
                g0 = tpool.tile([B, GC], fp32, tag="g0")
                nc.scalar.activation(g0[:], pg0[0:64, 0, :], AF.Copy)
                nc.vector.tensor_add(g0[:], g0[:], pg0[64:128, 1, :])
                h0m = tpool.tile([B, HC], fp32, tag="h0m")
                lstm_pointwise(g0, c0, h0m)

                # ---- transpose h0m -> [HC, B] bf16, AG -> h0f ----
                pt0 = p1pool.tile([P, P], fp32, tag="ptr", name="pt0")
                nc.tensor.transpose(pt0[:, 0:B], h0m[:], ident[0:B, 0:B])
                h0T = tpool.tile([P, B], bf16, tag="h0T")
                nc.vector.tensor_copy(h0T[:], pt0[:, 0:B])
                exchange(0, h0T[:], B, h0f[t % 2])

                # ---- gates1: bias + K = [h0(8) | h1(8)] ----
                h0f_w = h0f[t % 2]
                pg1 = ppool.tile([P, 2, GC], fp32, tag="pg")
                order1 = [8, 9, 10, 11, 12, 13, 14, 15] + [0, 1, 2, 3, 4, 5, 6, 7]
                nc.tensor.matmul(
                    pg1[0:B, 0, :], ones[:], gbb[:, GC : 2 * GC],
                    start=True, stop=False, tile_position=(0, 0),
                )
                for i, kt in enumerate(order1):
                    lhsT = (
                        h0f_w[:, kt * B : (kt + 1) * B]
                        if kt < 8
                        else h1f_r[:, (kt - 8) * B : (kt - 7) * B]
                    )
                    hf = (i + 1) % 2
                    nc.tensor.matmul(
                        pg1[64 * hf : 64 * hf + 64, hf, :],
                        lhsT,
                        w1T[:, kt, :],
                        start=(i < 1),
                        stop=(i >= 14),
                        tile_position=(0, 64 * hf),
                    )
                g1 = tpool.tile([B, GC], fp32, tag="g1")
                nc.scalar.activation(g1[:], pg1[0:64, 0, :], AF.Copy)
                nc.vector.tensor_add(g1[:], g1[:], pg1[64:128, 1, :])
                h1m = tpool.tile([B, HC], fp32, tag="h1m")
                lstm_pointwise(g1, c1, h1m)

                # ---- transpose h1m, AG -> h1f ----
                pt1 = p1pool.tile([P, P], fp32, tag="ptr", name="pt1")
                nc.tensor.transpose(pt1[:, 0:B], h1m[:], ident[0:B, 0:B])
                h1T = tpool.tile([P, B], bf16, tag="h1T")
                nc.vector.tensor_copy(h1T[:], pt1[:, 0:B])
                exchange(1, h1T[:], B, h1f[t % 2])

                # ---- select my batch columns of h1 (query) ----
                h1f_wv = h1f[t % 2][:].rearrange("p (kc b) -> p kc b", kc=8)
                nc.vector.tensor_copy(h1my[:], h1f_wv[:, :, bass.ts(pid, BG)])

                # ---- scores: per-b matvec via tile_position packing ----
                psc = p1pool.tile([P, 2, S], fp32, tag="psc")
                nc.vector.memset(psc[:], 0.0)
                for j in range(BG):
                    half, row = j // 4, 32 * (j % 4)
                    for kt in range(8):
                        nc.tensor.matmul(
                            psc[row : row + 1, half, :],
                            h1my[:, kt, j : j + 1],
                            m1t[:, j, kt, :],
                            start=(kt == 0),
                            stop=(kt == 7),
                            tile_position=(0, row),
                        )
                # ---- softmax over the two halves (garbage rows are fine) ----
                a_sb = tpool.tile([P, 2, S], fp32, tag="a_sb")
                stat = tpool.tile([P, 4], fp32, tag="stat")
                for half in range(2):
                    nc.vector.tensor_add(
                        a_sb[:, half, :], psc[:, half, :], c1t[:, half, :]
                    )
                    nm = stat[:, 2 * half : 2 * half + 1]
                    nc.vector.tensor_reduce(
                        nm, a_sb[:, half, :], axis=AX.X, op=mybir.AluOpType.max,
                        negate=True,
                    )
                    sm = stat[:, 2 * half + 1 : 2 * half + 2]
                    nc.scalar.activation(
                        a_sb[:, half, :], a_sb[:, half, :], AF.Exp, bias=nm,
                        accum_out=sm,
                    )
                    nc.vector.reciprocal(sm, sm)
                    nc.vector.tensor_scalar_mul(a_sb[:, half, :], a_sb[:, half, :], sm)

                # ---- transpose a -> columns; build block-diag lhsT (bf16) ----
                paT = p1pool.tile([P, 2, S], fp32, tag="psc", name="paT")
                nc.tensor.transpose(paT[:, 0, :], a_sb[:, 0, :], ident[:])
                nc.tensor.transpose(paT[:, 1, :], a_sb[:, 1, :], ident[:])
                abd = tpool.tile([P, BG * BG], bf16, tag="abd")
                nc.vector.memset(abd[:], 0.0)
                nc.vector.tensor_copy(
                    abd[:, 0 : BG * BG : 9].rearrange("p (a b) -> p a b", a=2),
                    paT[:].rearrange("p h (c x) -> p h c x", c=4)[:, :, :, 0:1],
                )

                # ---- z = b2 + blockdiag(a) @ M2stack + h1my.T @ W2h.T ----
                pz = p1pool.tile([BG, E], fp32, tag="pz")
                nc.tensor.matmul(
                    pz[:], ones[:, 0:BG], gbb[:, 2 * GC :], start=True, stop=False
                )
                for j in range(BG):
                    nc.tensor.matmul(
                        pz[:],
                        abd[:, j * BG : (j + 1) * BG],
                        m2s[:, j, :],
                        start=False,
                        stop=False,
                    )
                for kt in range(8):
                    nc.tensor.matmul(
                        pz[:], h1my[:, kt, :], w2hb[:, kt, :], start=False,
                        stop=(kt == 7),
                    )
                o_sb = tpool.tile([BG, E], fp32, tag="o_sb")
                nc.scalar.activation(o_sb[:], pz[:], AF.Tanh)

                # ---- write output (int16) ----
                oq = tpool.tile([BG, E], i16, tag="oq")
                nc.scalar.activation(oq[:], o_sb[:], AF.Copy, scale=O_SCALE)
                nc.scalar.dma_start(d_out[t], oq[:])

                # ---- transpose o chunks -> [P, 4, BG] bf16, AG -> of ----
                poT = p1pool.tile([P, 4, BG], fp32, tag="ptr", name="poT")
                for cchunk in range(4):
                    nc.tensor.transpose(
                        poT[:, cchunk, :],
                        o_sb[:, cchunk * P : (cchunk + 1) * P],
                        ident[0:BG, 0:BG],
                    )
                oT = tpool.tile([P, 4 * BG], bf16, tag="oT")
                nc.vector.tensor_copy(
                    oT[:].rearrange("p (c j) -> p c j", c=4), poT[:]
                )
                exchange(2, oT[:], 4 * BG, of[t % 2])

    nc.compile()
    return nc


def _scale_of(x, bits):
    m = float((1 << (bits - 1)) - 1)
    s = max(float(x.max()), -float(x.min()), 0.0)
    return s / m if s > 0 else 1.0


def _quant_to(x, s, dtype):
    buf = x * np.float32(1.0 / s)
    np.rint(buf, out=buf)
    return buf.astype(dtype)


def _host_prep_globals(inputs: dict, n_steps: int, put=None):
    """Compute global (concatenated) device inputs; when `put` is given the
    big arrays are dispatched to the devices as soon as they are ready so the
    tunnel transfer overlaps the remaining host prep."""
    f32, i16, i8 = np.float32, np.int16, np.int8
    n_pad = ((n_steps + NCORES - 1) // NCORES) * NCORES
    n_x, off_h0, off_h1, off_oi, nb1 = _blob_layout(n_pad)
    g = {}

    tgt = np.asarray(inputs["tgt_batch"])
    h_enc = np.asarray(inputs["h_encoder"], f32)
    emb = np.asarray(inputs["emb"], f32)
    out_init = np.asarray(inputs["output_init"], f32)
    hid_init = np.asarray(inputs["hidden_init"], f32)
    W_ih = np.asarray(inputs["W_ih"], f32)
    W_hh = np.asarray(inputs["W_hh"], f32)
    b_ih = np.asarray(inputs["b_ih"], f32)
    b_hh = np.asarray(inputs["b_hh"], f32)
    W1 = np.asarray(inputs["W1"], f32)
    b1v = np.asarray(inputs["b1"], f32)
    W2 = np.asarray(inputs["W2"], f32)
    b2v = np.asarray(inputs["b2"], f32)

    # LSTM weights: int8, gate-dim sharded, natural [rows, k_in] layout
    s_w = max(_scale_of(W_ih, 8), _scale_of(W_hh, 8))
    Wcat = np.concatenate([W_ih, W_hh], axis=2)  # [2, 4096, 2048]
    lw = np.ascontiguousarray(
        _quant_to(Wcat, s_w, i8).reshape(2, 4, 8, P, 2048).transpose(2, 0, 1, 3, 4)
    ).reshape(NCORES * L, GC, 2048)
    g["lw"] = put(lw) if put else lw

    # h_encoder: int16, batch-sharded, natural [S, V] layout
    s_h = _scale_of(h_enc, 16)
    g_henc = _quant_to(h_enc, s_h, i16).reshape(NCORES * BG, S, V)
    g["henc"] = put(g_henc) if put else g_henc

    # x sequence: int16, feature-major [T, P, 4, B], T-sharded
    xs = emb[tgt[:n_steps]]  # [n, B, E]
    s_x = _scale_of(xs, 16)
    xq = _quant_to(xs, s_x, i16)
    if n_pad != n_steps:
        xq = np.concatenate([xq, np.zeros((n_pad - n_steps, B, E), i16)], axis=0)
    xfold = np.ascontiguousarray(
        xq.transpose(0, 2, 1).reshape(n_pad, 4, P, B).transpose(0, 2, 1, 3)
    )  # [n_pad, P, 4, B]

    # W1 / W2.T: int16, row-sharded
    s_w1 = _scale_of(W1, 16)
    w1s = _quant_to(W1, s_w1, i16).reshape(NCORES, P, H)
    s_w2 = _scale_of(W2, 16)
    w2s = np.ascontiguousarray(_quant_to(W2, s_w2, i16).T).reshape(NCORES, 2 * P, E)

    # state inits: int16, feature-chunk sharded
    s_hi = _scale_of(hid_init, 16)
    h0s = _quant_to(hid_init[0].T, s_hi, i16).reshape(NCORES, P, B)
    h1s = _quant_to(hid_init[1].T, s_hi, i16).reshape(NCORES, P, B)
    s_oi = _scale_of(out_init, 16)
    oi4 = _quant_to(out_init.T, s_oi, i16)
    ois = np.ascontiguousarray(
        oi4.reshape(4, P, NCORES, BG).transpose(2, 1, 0, 3)
    )  # [k, P, 4, BG]

    # blob assembly
    blob = np.empty((NCORES, nb1), i16)
    blob[:, OFF_W1 : OFF_W1 + N_W1] = w1s.reshape(NCORES, -1)
    blob[:, OFF_W2 : OFF_W2 + N_W2] = w2s.reshape(NCORES, -1)
    blob[:, OFF_X : OFF_X + n_x] = xfold.reshape(NCORES, -1)
    blob[:, off_h0 : off_h0 + N_HI] = h0s.reshape(NCORES, -1)
    blob[:, off_h1 : off_h1 + N_HI] = h1s.reshape(NCORES, -1)
    blob[:, off_oi : off_oi + N_OI] = ois.reshape(NCORES, -1)
    blob = blob.reshape(-1)
    g["blob"] = put(blob) if put else blob

    # c1 rows (host fp32, exact): c1[b] = h_enc[b] @ b1
    c1v = (h_enc.reshape(-1, V) @ b1v).reshape(B, S).astype(f32)
    g["c1r"] = np.ascontiguousarray(c1v)  # [B, S] == [8*BG, S]

    # gate biases + b2
    b01 = (b_ih + b_hh).reshape(2, 4, NCORES, P).transpose(2, 0, 1, 3).reshape(
        NCORES, 2, GC
    )
    g_gb = np.empty((NCORES, 3, GC), f32)
    g_gb[:, 0:2] = b01
    g_gb[:, 2] = b2v
    g["gb"] = np.ascontiguousarray(g_gb).reshape(NCORES * 3, GC)

    # scales [P, 8]: s_w, s_x, s_hi, s_oi, s_h*s_w1, s_h*s_w2, s_w2, 0
    srow = np.array(
        [s_w, s_x, s_hi, s_oi, s_h * s_w1, s_h * s_w2, s_w2, 0.0], f32
    )
    g["sc"] = np.broadcast_to(srow, (NCORES * P, 8)).copy()
    return g


def _host_prep(inputs: dict, n_steps: int):
    """Per-core in_maps view (used by the sim harness)."""
    g = _host_prep_globals(inputs, n_steps)
    n_pad = ((n_steps + NCORES - 1) // NCORES) * NCORES
    nb1 = _blob_layout(n_pad)[-1]
    per = {
        "blob": g["blob"].reshape(NCORES, nb1),
        "lw": g["lw"].reshape(NCORES, L, GC, 2048),
        "henc": g["henc"].reshape(NCORES, BG, S, V),
        "c1r": g["c1r"].reshape(NCORES, BG, S),
        "gb": g["gb"].reshape(NCORES, 3, GC),
        "sc": g["sc"].reshape(NCORES, P, 8),
    }
    return [{name: arr[k] for name, arr in per.items()} for k in range(NCORES)]


def _assemble(outs, n_steps):
    """outs: list of per-core [n, BG, E] int16 -> [B, n, E] fp32."""
    full = np.stack(outs, axis=0)  # [k, n, BG, E]
    full = full.transpose(0, 2, 1, 3).reshape(B, n_steps, E)
    return full.astype(np.float32) * np.float32(1.0 / O_SCALE)


def _get_exec(n_steps: int):
    if n_steps in _CACHE:
        return _CACHE[n_steps]
    import jax
    import jax.numpy as jnp
    from jax.sharding import NamedSharding
    from concourse import bass2jax
    import concourse.mybir as mybir

    nc = _build(n_steps)
    bass2jax.install_neuronx_cc_hook()

    partition_name = nc.partition_id_tensor.name if nc.partition_id_tensor else None
    in_names, out_names, out_avals = [], [], []
    for alloc in nc.m.functions[0].allocations:
        if not isinstance(alloc, mybir.MemoryLocationSet):
            continue
        name = alloc.memorylocations[0].name
        if alloc.kind == "ExternalInput":
            if name != partition_name:
                in_names.append(name)
        elif alloc.kind == "ExternalOutput":
            out_names.append(name)
            out_avals.append(
                jax.core.ShapedArray(
                    tuple(alloc.tensor_shape), mybir.dt.np(alloc.dtype)
                )
            )
    n_params = len(in_names)
    all_names = list(in_names) + list(out_names)
    if partition_name is not None:
        all_names.append(partition_name)

    def _body(*args):
        operands = list(args)
        if partition_name is not None:
            operands.append(bass2jax.partition_id_tensor())
        outs = bass2jax._bass_exec_p.bind(
            *operands,
            out_avals=tuple(out_avals),
            in_names=tuple(all_names),
            out_names=tuple(out_names),
            lowering_input_output_aliases=(),
            sim_require_finite=True,
            sim_require_nnan=True,
            nc=nc,
        )
        return tuple(outs)

    devices = jax.devices()[:NCORES]
    mesh = bass2jax.Mesh(np.asarray(devices), ("core",))
    PS = bass2jax.PartitionSpec
    in_specs = (PS("core"),) * (n_params + len(out_names))
    out_specs = (PS("core"),) * len(out_names)
    donate = tuple(range(n_params, n_params + len(out_names)))
    sharded = jax.jit(
        bass2jax.shard_map(
            _body, mesh=mesh, in_specs=in_specs, out_specs=out_specs, check_rep=False
        ),
        donate_argnums=donate,
        keep_unused=True,
    )
    shardings = tuple(NamedSharding(mesh, PS("core")) for _ in out_avals)
    gshapes = [(NCORES * a.shape[0], *a.shape[1:]) for a in out_avals]
    gdtypes = [a.dtype for a in out_avals]
    zfn = jax.jit(
        lambda: tuple(jnp.zeros(s, d) for s, d in zip(gshapes, gdtypes)),
        out_shardings=shardings,
    )
    state = {
        "sharded": sharded,
        "zfn": zfn,
        "in_names": in_names,
        "out_names": out_names,
        "out_avals": out_avals,
        "nc": nc,
        "sharding": NamedSharding(mesh, PS("core")),
    }
    _CACHE[n_steps] = state
    return state


def run(inputs: dict, n_steps: int = T):
    import jax

    st = _get_exec(n_steps)
    zeros = st["zfn"]()  # async device-side zeros
    sharding = st["sharding"]
    put = lambda arr: jax.device_put(arr, sharding)
    g = _host_prep_globals(inputs, n_steps, put=put)
    outs = st["sharded"](*[g[n] for n in st["in_names"]], *zeros)
    oname_i = st["out_names"].index("out")
    o = np.asarray(outs[oname_i])  # [8*n, BG, E] int16
    o = o.reshape(NCORES, n_steps, BG, E)
    return _assemble(list(o), n_steps)


def kernel(**inputs) -> np.ndarray:
    return run(inputs, T)


# revision 26
# speedup vs baseline: 10.0529x; 1.2101x over previous
"""AttentionDecoder Trainium2 kernel: 8-way model-parallel LSTM+attention decoder.

v2 — optimized for end-to-end wall clock through the axon tunnel (~82MB/s):
  - Quantized transfer: LSTM weights int8, h_encoder/W1/W2/xseq int16,
    output int16 (~43MB/call vs 210MB fp32 baseline).
  - Attention precompute (M1T = (h_enc @ W1).T, M2 = h_enc @ W2v.T) moved
    from host numpy (was 4.3s) onto the device prologue (fp32 PE matmuls
    on exact int16 operands, scales folded into the PSUM->SBUF copies).
  - Shared tensors (W1, W2, xseq, state inits) shipped as 1/8 shards and
    AllGathered on device.
  - All layout transposes on device (PE transpose / DMA XBAR transpose);
    host prep is quantize + contiguous reshapes only.
  - Steady-state exec path caches the jitted shard_map wrapper (no per-call
    retrace) and creates donated output zeros on device.
  - Step loop matmuls in bf16 (weights already <= 8-bit precision),
    pointwise/softmax in fp32, per-step AllGathers in bf16.

Numerics validated vs reference: rel err ~2.8e-3 (tolerance 2e-2).

Layout summary (per core k):
  - Weights sharded over the 4H gate dim: core k owns gate rows
    [g*H + k*128, g*H + (k+1)*128) for g in 0..3 of both layers.
  - Activations feature-major [feat, batch]; gates matmuls are
    lhsT = activation chunk [K=128 feats, M=B], rhs = weight.T chunk.
  - Per timestep: 3 bf16 AllGathers (h0, h1, o) across the 8 cores.
  - Attention per-core batch shard: core k owns batches 8k..8k+7.
"""

import warnings

warnings.filterwarnings("ignore")

import numpy as np

VOCAB, E, H, L, B, T, S, V = 32000, 512, 1024, 2, 64, 64, 128, 1024
NCORES = 8
P = 128
BG = B // NCORES  # 8 batches per core for attention
HC = H // NCORES  # 128 hidden feats per core
GC = 4 * HC  # 512 gate rows per core

O_SCALE = 32000.0  # fixed output quant scale (tanh output, |o| <= 1)

# ---- AllGather blob layout (int16 elems, per-core contribution) ----
N_W1 = P * H  # [128, 1024] W1 row chunk
N_W2 = 2 * P * E  # [256, 512] W2.T row chunk
N_HI = P * B  # [128, 64] hidden-init feature chunk
N_OI = P * 4 * BG  # [128, 4, 8] output-init chunk
OFF_W1 = 0
OFF_W2 = OFF_W1 + N_W1
OFF_X = OFF_W2 + N_W2


def _blob_layout(n_pad):
    n_x = (n_pad // NCORES) * P * 4 * B
    off_h0 = OFF_X + n_x
    off_h1 = off_h0 + N_HI
    off_oi = off_h1 + N_HI
    nb1 = off_oi + N_OI
    return n_x, off_h0, off_h1, off_oi, nb1


_CACHE = {}


def _build(n_steps: int):
    import concourse.bass as bass
    import concourse.bacc as bacc
    import concourse.mybir as mybir
    import concourse.tile as tile
    from concourse import masks

    fp32 = mybir.dt.float32
    bf16 = mybir.dt.bfloat16
    i16 = mybir.dt.int16
    i8 = mybir.dt.int8
    AF = mybir.ActivationFunctionType
    AX = mybir.AxisListType

    n_pad = ((n_steps + NCORES - 1) // NCORES) * NCORES
    n_x, off_h0, off_h1, off_oi, nb1 = _blob_layout(n_pad)
    x_per_core = (n_pad // NCORES) * P * 4 * B

    nc = bacc.Bacc("TRN2", target_bir_lowering=False, debug=False, num_devices=NCORES)

    # ---- DRAM I/O ----
    d_blob = nc.dram_tensor("blob", [nb1], i16, kind="ExternalInput")
    d_lw = nc.dram_tensor("lw", [L, GC, 2048], i8, kind="ExternalInput")
    d_henc = nc.dram_tensor("henc", [BG, S, V], i8, kind="ExternalInput")
    d_c1r = nc.dram_tensor("c1r", [BG, S], fp32, kind="ExternalInput")
    d_gb = nc.dram_tensor("gb", [3, GC], fp32, kind="ExternalInput")
    d_sc = nc.dram_tensor("sc", [P, 8], fp32, kind="ExternalInput")
    d_out = nc.dram_tensor("out", [n_steps, BG, E], i16, kind="ExternalOutput")

    RG = [list(range(NCORES))]

    with tile.TileContext(nc) as tc:
        import contextlib

        ctx = contextlib.ExitStack()
        with ctx:
            wpool = ctx.enter_context(tc.tile_pool(name="weights", bufs=1))
            spool = ctx.enter_context(tc.tile_pool(name="state", bufs=1))
            propool = ctx.enter_context(tc.tile_pool(name="pro", bufs=1))
            xpool = ctx.enter_context(tc.tile_pool(name="x", bufs=2))
            tpool = ctx.enter_context(tc.tile_pool(name="tmp", bufs=2))
            ppool = ctx.enter_context(tc.tile_pool(name="psum", bufs=2, space="PSUM"))
            p1pool = ctx.enter_context(tc.tile_pool(name="psum1", bufs=1, space="PSUM"))
            dpool = ctx.enter_context(tc.tile_pool(name="dram", bufs=2, space="DRAM"))
            d1pool = ctx.enter_context(tc.tile_pool(name="dram1", bufs=1, space="DRAM"))

            # ---- persistent SBUF tiles ----
            w0T = wpool.tile([P, 16, GC], bf16, tag="w0T")
            w1T = wpool.tile([P, 16, GC], bf16, tag="w1T")
            gbb = wpool.tile([1, 3 * GC], bf16, tag="gbb")
            m1t = wpool.tile([P, BG, 8, S], bf16, tag="m1t")
            c1t = wpool.tile([P, 2, S], fp32, tag="c1t")
            m2s = wpool.tile([P, BG, E], bf16, tag="m2s")
            w2hb = wpool.tile([P, 8, E], bf16, tag="w2hb")
            ident = wpool.tile([P, P], fp32, tag="ident")
            ones = wpool.tile([1, B], bf16, tag="ones")
            sc = wpool.tile([P, 8], fp32, tag="sc")

            h0f = [
                spool.tile([P, NCORES * B], bf16, tag=f"h0f{i}", name=f"h0f{i}")
                for i in range(2)
            ]
            h1f = [
                spool.tile([P, NCORES * B], bf16, tag=f"h1f{i}", name=f"h1f{i}")
                for i in range(2)
            ]
            of = [
                spool.tile([P, NCORES * 4 * BG], bf16, tag=f"of{i}", name=f"of{i}")
                for i in range(2)
            ]
            c0 = spool.tile([B, HC], fp32, tag="c0")
            c1 = spool.tile([B, HC], fp32, tag="c1")
            h1my = spool.tile([P, 8, BG], bf16, tag="h1my")

            # ---- prologue transients ----
            w1f = propool.tile([P, 8, H], fp32, tag="w1f")
            w2vf = propool.tile([P, 8, E], fp32, tag="w2vf")
            lw8 = propool.tile([P, 4, 2048], i8, tag="lw8")
            castbuf = propool.tile([P, 2048], fp32, tag="castbuf")
            st16 = propool.tile([P, H], i16, tag="st16")
            he8 = propool.tile([P, V], i8, tag="he8")
            h16f = propool.tile([P, 8, S], fp32, tag="h16f")
            his16 = propool.tile([P, NCORES, B], i16, tag="his16")
            ois16 = propool.tile([P, NCORES * 4 * BG], i16, tag="ois16")

            # ---- basics ----
            nc.sync.dma_start(sc[:], d_sc[:])
            gbf = propool.tile([1, 3 * GC], fp32, tag="gbf")
            nc.sync.dma_start(
                gbf[:], d_gb[:].rearrange("l g -> (l g)").rearrange("(a x) -> a x", a=1)
            )
            nc.vector.tensor_copy(gbb[:], gbf[:])
            nc.vector.memset(ones[:], 1.0)
            masks.make_identity(nc, ident[:])
            nc.vector.memset(c0[:], 0.0)
            nc.vector.memset(c1[:], 0.0)

            pid = nc.vector.partition_id()

            # ---- blob AllGather ----
            agi = d1pool.tile([nb1], i16, tag="agi")
            ago = d1pool.tile([NCORES, nb1], i16, tag="ago")
            nc.sync.dma_start(agi[:], d_blob[:])
            nc.gpsimd.collective_compute(
                "AllGather",
                mybir.AluOpType.bypass,
                replica_groups=RG,
                ins=[agi.opt()],
                outs=[ago.opt()],
            )

            # ---- W1 -> w1f fp32 [p, vc, h] (int-valued) ----
            for vc in range(NCORES):
                nc.sync.dma_start(
                    st16[:],
                    ago[vc, OFF_W1 : OFF_W1 + N_W1].rearrange("(p h) -> p h", p=P),
                )
                nc.vector.tensor_copy(w1f[:, vc, :], st16[:])

            # ---- W2.T chunks: vc 0..7 -> w2vf fp32 int-valued;
            #      hc 0..7 -> w2hb bf16 real-valued (scale s_w2) ----
            for rc in range(16):
                k, half = rc // 2, rc % 2
                src = ago[
                    k, OFF_W2 + half * P * E : OFF_W2 + (half + 1) * P * E
                ].rearrange("(p e) -> p e", p=P)
                nc.scalar.dma_start(st16[:, 0:E], src)
                if rc < 8:
                    nc.vector.tensor_copy(w2vf[:, rc, :], st16[:, 0:E])
                else:
                    nc.scalar.activation(
                        w2hb[:, rc - 8, :], st16[:, 0:E], AF.Copy, scale=sc[:, 6:7]
                    )

            # ---- LSTM weights: int8 -> cast -> PE transpose -> scaled bf16 ----
            for l in range(L):
                wT = w0T if l == 0 else w1T
                nc.sync.dma_start(
                    lw8[:], d_lw[l].rearrange("(c p) k -> p c k", p=P)
                )
                for c in range(4):
                    nc.vector.tensor_copy(castbuf[:], lw8[:, c, :])
                    for kb in range(16):
                        ptw = ppool.tile([P, 2, GC], fp32, tag="pg", name=f"ptw{l}_{c}_{kb}")
                        nc.tensor.transpose(
                            ptw[:, 0, 0:P],
                            castbuf[:, kb * P : (kb + 1) * P],
                            ident[:],
                        )
                        nc.scalar.activation(
                            wT[:, kb, c * P : (c + 1) * P],
                            ptw[:, 0, 0:P],
                            AF.Copy,
                            scale=sc[:, 0:1],
                        )

            # ---- h_enc (int8): cast + PE transpose + m1t/m2s (scales folded) ----
            for j in range(BG):
                nc.sync.dma_start(he8[:], d_henc[j])
                nc.vector.tensor_copy(castbuf[:, 0:V], he8[:])
                for vc in range(8):
                    pht = ppool.tile([P, 2, GC], fp32, tag="pg", name=f"pht{j}_{vc}")
                    nc.tensor.transpose(
                        pht[:, 0, 0:P], castbuf[:, vc * P : (vc + 1) * P], ident[:]
                    )
                    nc.vector.tensor_copy(h16f[:, vc, :], pht[:, 0, 0:P])
                for kt in range(8):
                    pm = ppool.tile([P, 2, GC], fp32, tag="pg", name=f"pm{j}_{kt}")
                    for vc in range(8):
                        nc.tensor.matmul(
                            pm[:, 0, 0:S],
                            w1f[:, vc, kt * P : (kt + 1) * P],
                            h16f[:, vc, :],
                            start=(vc == 0),
                            stop=(vc == 7),
                        )
                    nc.scalar.activation(
                        m1t[:, j, kt, :], pm[:, 0, 0:S], AF.Copy, scale=sc[:, 4:5]
                    )
                pm2 = ppool.tile([P, 2, GC], fp32, tag="pg", name=f"pm2_{j}")
                for vc in range(8):
                    nc.tensor.matmul(
                        pm2[:, 0, :],
                        h16f[:, vc, :],
                        w2vf[:, vc, :],
                        start=(vc == 0),
                        stop=(vc == 7),
                    )
                nc.scalar.activation(
                    m2s[:, j, :], pm2[:, 0, :], AF.Copy, scale=sc[:, 5:6]
                )

            # ---- state inits from blob ----
            nc.sync.dma_start(
                his16[:], ago[:, off_h0 : off_h0 + N_HI].rearrange("k (p b) -> p k b", p=P)
            )
            nc.scalar.activation(
                h0f[1][:],
                his16[:].rearrange("p k b -> p (k b)"),
                AF.Copy,
                scale=sc[:, 2:3],
            )
            nc.sync.dma_start(
                his16[:], ago[:, off_h1 : off_h1 + N_HI].rearrange("k (p b) -> p k b", p=P)
            )
            nc.scalar.activation(
                h1f[1][:],
                his16[:].rearrange("p k b -> p (k b)"),
                AF.Copy,
                scale=sc[:, 2:3],
            )
            nc.sync.dma_start(
                ois16[:].rearrange("p (k c j) -> p k c j", k=NCORES, c=4),
                ago[:, off_oi : off_oi + N_OI].rearrange(
                    "k (p c j) -> p k c j", p=P, c=4
                ),
            )
            nc.scalar.activation(of[1][:], ois16[:], AF.Copy, scale=sc[:, 3:4])

            # ---- c1t rows ----
            nc.vector.memset(c1t[:], 0.0)
            for j in range(BG):
                nc.scalar.dma_start(
                    c1t[32 * (j % 4) : 32 * (j % 4) + 1, j // 4, :],
                    d_c1r[j].rearrange("(a s) -> a s", a=1),
                )

            def lstm_pointwise(g_sb, cst, h_out):
                """g_sb [B, 4*HC] gates i,f,g,o; updates cst, writes h_out [B,HC]."""
                gt = tpool.tile([B, HC], fp32, tag="pw_gt")
                ot = tpool.tile([B, HC], fp32, tag="pw_ot")
                ift = tpool.tile([B, 2 * HC], fp32, tag="pw_ift")
                nc.scalar.activation(ift[:], g_sb[:, 0 : 2 * HC], AF.Sigmoid)
                it, ft = ift[:, 0:HC], ift[:, HC : 2 * HC]
                nc.scalar.activation(gt[:], g_sb[:, 2 * HC : 3 * HC], AF.Tanh)
                nc.scalar.activation(ot[:], g_sb[:, 3 * HC : 4 * HC], AF.Sigmoid)
                t1 = tpool.tile([B, HC], fp32, tag="pw_t1")
                nc.vector.tensor_mul(t1[:], ft, cst[:])
                nc.vector.tensor_mul(gt[:], it, gt[:])
                nc.vector.tensor_add(cst[:], t1[:], gt[:])
                tc_ = tpool.tile([B, HC], fp32, tag="pw_tc")
                nc.scalar.activation(tc_[:], cst[:], AF.Tanh)
                nc.vector.tensor_mul(h_out[:], ot[:], tc_[:])

            def exchange(kind, src_sb, width, dst_tile):
                """Broadcast my [P,width] bf16 chunk into slot k of everyone's dst."""
                bi = dpool.tile([P, width], bf16, tag=f"agi{kind}", name=f"agi{kind}")
                bo = dpool.tile(
                    [P * NCORES, width], bf16, tag=f"ago{kind}", name=f"ago{kind}"
                )
                nc.gpsimd.dma_start(bi[:], src_sb)
                nc.gpsimd.collective_compute(
                    "AllGather",
                    mybir.AluOpType.bypass,
                    replica_groups=RG,
                    ins=[bi.opt()],
                    outs=[bo.opt()],
                )
                nc.gpsimd.dma_start(
                    dst_tile[:].rearrange("p (k w) -> p k w", k=NCORES),
                    bo[:].rearrange("(k p) w -> p k w", p=P),
                )

            x_step = P * 4 * B

            for t in range(n_steps):
                # ---- x load (int16 from AG'd blob) + dequant to bf16 ----
                xi16 = xpool.tile([P, 4, B], i16, tag="xi16")
                kc, tt = t // (n_pad // NCORES), t % (n_pad // NCORES)
                nc.scalar.dma_start(
                    xi16[:],
                    ago[kc, OFF_X + tt * x_step : OFF_X + (tt + 1) * x_step].rearrange(
                        "(p c b) -> p c b", p=P, c=4
                    ),
                )
                xt = xpool.tile([P, 4, B], bf16, tag="xt")
                nc.scalar.activation(xt[:], xi16[:], AF.Copy, scale=sc[:, 1:2])

                h0f_r = h0f[(t - 1) % 2]
                h1f_r = h1f[(t - 1) % 2]
                of_r = of[(t - 1) % 2]
                of_rv = of_r[:].rearrange("p (k c j) -> p c k j", k=NCORES, c=4)
                o4 = tpool.tile([P, 4, B], bf16, tag="o4")
                nc.vector.tensor_copy(
                    o4[:].rearrange("p c (k j) -> p c k j", k=NCORES), of_rv
                )

                # ---- gates0: bias + K = [x(4) | o(4) | h0(8)] ----
                pg0 = ppool.tile([P, 2, GC], fp32, tag="pg")
                order0 = [0, 1, 2, 3] + [8, 9, 10, 11, 12, 13, 14, 15] + [4, 5, 6, 7]
                nc.tensor.matmul(
                    pg0[0:B, 0, :], ones[:], gbb[:, 0:GC],
                    start=True, stop=False, tile_position=(0, 0),
                )
                for i, kt in enumerate(order0):
                    if kt < 4:
                        lhsT = xt[:, kt, :]
                    elif kt < 8:
                        lhsT = o4[:, kt - 4, :]
                    else:
                        lhsT = h0f_r[:, (kt - 8) * B : (kt - 7) * B]
                    hf = (i + 1) % 2
                    nc.tensor.matmul(
                        pg0[64 * hf : 64 * hf + 64, hf, :],
                        lhsT,
                        w0T[:, kt, :],
                        start=(i < 1),
                        stop=(i >= 14),
                        tile_position=(0, 64 * hf),
                    )
                g0 = tpool.tile([B, GC], fp32, tag="g0")
                nc.scalar.activation(g0[:], pg0[0:64, 0, :], AF.Copy)
                nc.vector.tensor_add(g0[:], g0[:], pg0[64:128, 1, :])
                h0m = tpool.tile([B, HC], fp32, tag="h0m")
                lstm_pointwise(g0, c0, h0m)

                # ---- transpose h0m -> [HC, B] bf16, AG -> h0f ----
                pt0 = p1pool.tile([P, P], fp32, tag="ptr", name="pt0")
                nc.tensor.transpose(pt0[:, 0:B], h0m[:], ident[0:B, 0:B])
                h0T = tpool.tile([P, B], bf16, tag="h0T")
                nc.vector.tensor_copy(h0T[:], pt0[:, 0:B])
                exchange(0, h0T[:], B, h0f[t % 2])

                # ---- gates1: bias + K = [h0(8) | h1(8)] ----
                h0f_w = h0f[t % 2]
                pg1 = ppool.tile([P, 2, GC], fp32, tag="pg")
                order1 = [8, 9, 10, 11, 12, 13, 14, 15] + [0, 1, 2, 3, 4, 5, 6, 7]
                nc.tensor.matmul(
                    pg1[0:B, 0, :], ones[:], gbb[:, GC : 2 * GC],
                    start=True, stop=False, tile_position=(0, 0),
                )
                for i, kt in enumerate(order1):
                    lhsT = (
                        h0f_w[:, kt * B : (kt + 1) * B]
                        if kt < 8
                        else h1f_r[:, (kt - 8) * B : (kt - 7) * B]
                    )
                    hf = (i + 1) % 2
                    nc.tensor.matmul(
                        pg1[64 * hf : 64 * hf + 64, hf, :],
                        lhsT,
                        w1T[:, kt, :],
                        start=(i < 1),
                        stop=(i >= 14),
                        tile_position=(0, 64 * hf),
                    )
                g1 = tpool.tile([B, GC], fp32, tag="g1")
                nc.scalar.activation(g1[:], pg1[0:64, 0, :], AF.Copy)
                nc.vector.tensor_add(g1[:], g1[:], pg1[64:128, 1, :])
                h1m = tpool.tile([B, HC], fp32, tag="h1m")
                lstm_pointwise(g1, c1, h1m)

                # ---- transpose h1m, AG -> h1f ----
                pt1 = p1pool.tile([P, P], fp32, tag="ptr", name="pt1")
                nc.tensor.transpose(pt1[:, 0:B], h1m[:], ident[0:B, 0:B])
                h1T = tpool.tile([P, B], bf16, tag="h1T")
                nc.vector.tensor_copy(h1T[:], pt1[:, 0:B])
                exchange(1, h1T[:], B, h1f[t % 2])

                # ---- select my batch columns of h1 (query) ----
                h1f_wv = h1f[t % 2][:].rearrange("p (kc b) -> p kc b", kc=8)
                nc.vector.tensor_copy(h1my[:], h1f_wv[:, :, bass.ts(pid, BG)])

                # ---- scores: per-b matvec via tile_position packing ----
                psc = p1pool.tile([P, 2, S], fp32, tag="psc")
                nc.vector.memset(psc[:], 0.0)
                for j in range(BG):
                    half, row = j // 4, 32 * (j % 4)
                    for kt in range(8):
                        nc.tensor.matmul(
                            psc[row : row + 1, half, :],
                            h1my[:, kt, j : j + 1],
                            m1t[:, j, kt, :],
                            start=(kt == 0),
                            stop=(kt == 7),
                            tile_position=(0, row),
                        )
                # ---- softmax over the two halves (garbage rows are fine) ----
                a_sb = tpool.tile([P, 2, S], fp32, tag="a_sb")
                stat = tpool.tile([P, 4], fp32, tag="stat")
                for half in range(2):
                    nc.vector.tensor_add(
                        a_sb[:, half, :], psc[:, half, :], c1t[:, half, :]
                    )
                    nm = stat[:, 2 * half : 2 * half + 1]
                    nc.vector.tensor_reduce(
                        nm, a_sb[:, half, :], axis=AX.X, op=mybir.AluOpType.max,
                        negate=True,
                    )
                    sm = stat[:, 2 * half + 1 : 2 * half + 2]
                    nc.scalar.activation(
                        a_sb[:, half, :], a_sb[:, half, :], AF.Exp, bias=nm,
                        accum_out=sm,
                    )
                    nc.vector.reciprocal(sm, sm)
                    nc.vector.tensor_scalar_mul(a_sb[:, half, :], a_sb[:, half, :], sm)

                # ---- transpose a -> columns; build block-diag lhsT (bf16) ----
                paT = p1pool.tile([P, 2, S], fp32, tag="psc", name="paT")
                nc.tensor.transpose(paT[:, 0, :], a_sb[:, 0, :], ident[:])
                nc.tensor.transpose(paT[:, 1, :], a_sb[:, 1, :], ident[:])
                abd = tpool.tile([P, BG * BG], bf16, tag="abd")
                nc.vector.memset(abd[:], 0.0)
                nc.vector.tensor_copy(
                    abd[:, 0 : BG * BG : 9].rearrange("p (a b) -> p a b", a=2),
                    paT[:].rearrange("p h (c x) -> p h c x", c=4)[:, :, :, 0:1],
                )

                # ---- z = b2 + blockdiag(a) @ M2stack + h1my.T @ W2h.T ----
                pz = p1pool.tile([BG, E], fp32, tag="pz")
                nc.tensor.matmul(
                    pz[:], ones[:, 0:BG], gbb[:, 2 * GC :], start=True, stop=False
                )
                for j in range(BG):
                    nc.tensor.matmul(
                        pz[:],
                        abd[:, j * BG : (j + 1) * BG],
                        m2s[:, j, :],
                        start=False,
                        stop=False,
                    )
                for kt in range(8):
                    nc.tensor.matmul(
                        pz[:], h1my[:, kt, :], w2hb[:, kt, :], start=False,
                        stop=(kt == 7),
                    )
                o_sb = tpool.tile([BG, E], fp32, tag="o_sb")
                nc.scalar.activation(o_sb[:], pz[:], AF.Tanh)

                # ---- write output (int16) ----
                oq = tpool.tile([BG, E], i16, tag="oq")
                nc.scalar.activation(oq[:], o_sb[:], AF.Copy, scale=O_SCALE)
                nc.scalar.dma_start(d_out[t], oq[:])

                # ---- transpose o chunks -> [P, 4, BG] bf16, AG -> of ----
                poT = p1pool.tile([P, 4, BG], fp32, tag="ptr", name="poT")
                for cchunk in range(4):
                    nc.tensor.transpose(
                        poT[:, cchunk, :],
                        o_sb[:, cchunk * P : (cchunk + 1) * P],
                        ident[0:BG, 0:BG],
                    )
                oT = tpool.tile([P, 4 * BG], bf16, tag="oT")
                nc.vector.tensor_copy(
                    oT[:].rearrange("p (c j) -> p c j", c=4), poT[:]
                )
                exchange(2, oT[:], 4 * BG, of[t % 2])

    nc.compile()
    return nc


def _scale_of(x, bits):
    m = float((1 << (bits - 1)) - 1)
    s = max(float(x.max()), -float(x.min()), 0.0)
    return s / m if s > 0 else 1.0


def _quant_to(x, s, dtype):
    buf = x * np.float32(1.0 / s)
    np.rint(buf, out=buf)
    return buf.astype(dtype)


def _host_prep_globals(inputs: dict, n_steps: int, put=None, pool=None):
    """Compute global (concatenated) device inputs; when `put` is given the
    big arrays are dispatched to the devices as soon as they are ready so the
    tunnel transfer overlaps the remaining host prep. When `pool` is given the
    independent quantize passes run on worker threads (numpy releases the
    GIL), with puts dispatched in completion order."""
    f32, i16, i8 = np.float32, np.int16, np.int8
    n_pad = ((n_steps + NCORES - 1) // NCORES) * NCORES
    n_x, off_h0, off_h1, off_oi, nb1 = _blob_layout(n_pad)
    g = {}

    tgt = np.asarray(inputs["tgt_batch"])
    h_enc = np.asarray(inputs["h_encoder"], f32)
    emb = np.asarray(inputs["emb"], f32)
    out_init = np.asarray(inputs["output_init"], f32)
    hid_init = np.asarray(inputs["hidden_init"], f32)
    W_ih = np.asarray(inputs["W_ih"], f32)
    W_hh = np.asarray(inputs["W_hh"], f32)
    b_ih = np.asarray(inputs["b_ih"], f32)
    b_hh = np.asarray(inputs["b_hh"], f32)
    W1 = np.asarray(inputs["W1"], f32)
    b1v = np.asarray(inputs["b1"], f32)
    W2 = np.asarray(inputs["W2"], f32)
    b2v = np.asarray(inputs["b2"], f32)

    def prep_lw():
        # LSTM weights: int8, gate-dim sharded, natural [rows, k_in] layout
        s_w = max(_scale_of(W_ih, 8), _scale_of(W_hh, 8))
        Wcat = np.concatenate([W_ih, W_hh], axis=2)  # [2, 4096, 2048]
        lw = np.ascontiguousarray(
            _quant_to(Wcat, s_w, i8)
            .reshape(2, 4, 8, P, 2048)
            .transpose(2, 0, 1, 3, 4)
        ).reshape(NCORES * L, GC, 2048)
        return (put(lw) if put else lw), s_w

    def prep_henc():
        # h_encoder: int8, batch-sharded, natural [S, V] layout
        s_h = _scale_of(h_enc, 8)
        hq = _quant_to(h_enc, s_h, i8).reshape(NCORES * BG, S, V)
        return (put(hq) if put else hq), s_h

    if pool is not None:
        f_lw = pool.submit(prep_lw)
        f_henc = pool.submit(prep_henc)
    else:
        g["lw"], s_w = prep_lw()
        g["henc"], s_h = prep_henc()

    # x sequence: int16, feature-major [T, P, 4, B], T-sharded
    xs = emb[tgt[:n_steps]]  # [n, B, E]
    s_x = _scale_of(xs, 16)
    xq = _quant_to(xs, s_x, i16)
    if n_pad != n_steps:
        xq = np.concatenate([xq, np.zeros((n_pad - n_steps, B, E), i16)], axis=0)
    xfold = np.ascontiguousarray(
        xq.transpose(0, 2, 1).reshape(n_pad, 4, P, B).transpose(0, 2, 1, 3)
    )  # [n_pad, P, 4, B]

    # W1 / W2.T: int16, row-sharded
    s_w1 = _scale_of(W1, 16)
    w1s = _quant_to(W1, s_w1, i16).reshape(NCORES, P, H)
    s_w2 = _scale_of(W2, 16)
    w2s = np.ascontiguousarray(_quant_to(W2, s_w2, i16).T).reshape(NCORES, 2 * P, E)

    # state inits: int16, feature-chunk sharded
    s_hi = _scale_of(hid_init, 16)
    h0s = _quant_to(hid_init[0].T, s_hi, i16).reshape(NCORES, P, B)
    h1s = _quant_to(hid_init[1].T, s_hi, i16).reshape(NCORES, P, B)
    s_oi = _scale_of(out_init, 16)
    oi4 = _quant_to(out_init.T, s_oi, i16)
    ois = np.ascontiguousarray(
        oi4.reshape(4, P, NCORES, BG).transpose(2, 1, 0, 3)
    )  # [k, P, 4, BG]

    # blob assembly
    blob = np.empty((NCORES, nb1), i16)
    blob[:, OFF_W1 : OFF_W1 + N_W1] = w1s.reshape(NCORES, -1)
    blob[:, OFF_W2 : OFF_W2 + N_W2] = w2s.reshape(NCORES, -1)
    blob[:, OFF_X : OFF_X + n_x] = xfold.reshape(NCORES, -1)
    blob[:, off_h0 : off_h0 + N_HI] = h0s.reshape(NCORES, -1)
    blob[:, off_h1 : off_h1 + N_HI] = h1s.reshape(NCORES, -1)
    blob[:, off_oi : off_oi + N_OI] = ois.reshape(NCORES, -1)
    blob = blob.reshape(-1)
    g["blob"] = put(blob) if put else blob

    if pool is not None:
        g["lw"], s_w = f_lw.result()
        g["henc"], s_h = f_henc.result()

    # c1 rows (host fp32, exact): c1[b] = h_enc[b] @ b1
    c1v = (h_enc.reshape(-1, V) @ b1v).reshape(B, S).astype(f32)
    g["c1r"] = np.ascontiguousarray(c1v)  # [B, S] == [8*BG, S]

    # gate biases + b2
    b01 = (b_ih + b_hh).reshape(2, 4, NCORES, P).transpose(2, 0, 1, 3).reshape(
        NCORES, 2, GC
    )
    g_gb = np.empty((NCORES, 3, GC), f32)
    g_gb[:, 0:2] = b01
    g_gb[:, 2] = b2v
    g["gb"] = np.ascontiguousarray(g_gb).reshape(NCORES * 3, GC)

    # scales [P, 8]: s_w, s_x, s_hi, s_oi, s_h*s_w1, s_h*s_w2, s_w2, 0
    srow = np.array(
        [s_w, s_x, s_hi, s_oi, s_h * s_w1, s_h * s_w2, s_w2, 0.0], f32
    )
    g["sc"] = np.broadcast_to(srow, (NCORES * P, 8)).copy()
    return g


def _host_prep(inputs: dict, n_steps: int):
    """Per-core in_maps view (used by the sim harness)."""
    g = _host_prep_globals(inputs, n_steps)
    n_pad = ((n_steps + NCORES - 1) // NCORES) * NCORES
    nb1 = _blob_layout(n_pad)[-1]
    per = {
        "blob": g["blob"].reshape(NCORES, nb1),
        "lw": g["lw"].reshape(NCORES, L, GC, 2048),
        "henc": g["henc"].reshape(NCORES, BG, S, V),
        "c1r": g["c1r"].reshape(NCORES, BG, S),
        "gb": g["gb"].reshape(NCORES, 3, GC),
        "sc": g["sc"].reshape(NCORES, P, 8),
    }
    return [{name: arr[k] for name, arr in per.items()} for k in range(NCORES)]


def _assemble(outs, n_steps):
    """outs: list of per-core [n, BG, E] int16 -> [B, n, E] fp32."""
    full = np.stack(outs, axis=0)  # [k, n, BG, E]
    full = full.transpose(0, 2, 1, 3).reshape(B, n_steps, E)
    return full.astype(np.float32) * np.float32(1.0 / O_SCALE)


def _get_exec(n_steps: int):
    if n_steps in _CACHE:
        return _CACHE[n_steps]
    import jax
    import jax.numpy as jnp
    from jax.sharding import NamedSharding
    from concourse import bass2jax
    import concourse.mybir as mybir

    nc = _build(n_steps)
    bass2jax.install_neuronx_cc_hook()

    partition_name = nc.partition_id_tensor.name if nc.partition_id_tensor else None
    in_names, out_names, out_avals = [], [], []
    for alloc in nc.m.functions[0].allocations:
        if not isinstance(alloc, mybir.MemoryLocationSet):
            continue
        name = alloc.memorylocations[0].name
        if alloc.kind == "ExternalInput":
            if name != partition_name:
                in_names.append(name)
        elif alloc.kind == "ExternalOutput":
            out_names.append(name)
            out_avals.append(
                jax.core.ShapedArray(
                    tuple(alloc.tensor_shape), mybir.dt.np(alloc.dtype)
                )
            )
    n_params = len(in_names)
    all_names = list(in_names) + list(out_names)
    if partition_name is not None:
        all_names.append(partition_name)

    def _body(*args):
        operands = list(args)
        if partition_name is not None:
            operands.append(bass2jax.partition_id_tensor())
        outs = bass2jax._bass_exec_p.bind(
            *operands,
            out_avals=tuple(out_avals),
            in_names=tuple(all_names),
            out_names=tuple(out_names),
            lowering_input_output_aliases=(),
            sim_require_finite=True,
            sim_require_nnan=True,
            nc=nc,
        )
        return tuple(outs)

    devices = jax.devices()[:NCORES]
    mesh = bass2jax.Mesh(np.asarray(devices), ("core",))
    PS = bass2jax.PartitionSpec
    in_specs = (PS("core"),) * (n_params + len(out_names))
    out_specs = (PS("core"),) * len(out_names)
    donate = tuple(range(n_params, n_params + len(out_names)))
    sharded = jax.jit(
        bass2jax.shard_map(
            _body, mesh=mesh, in_specs=in_specs, out_specs=out_specs, check_rep=False
        ),
        donate_argnums=donate,
        keep_unused=True,
    )
    shardings = tuple(NamedSharding(mesh, PS("core")) for _ in out_avals)
    gshapes = [(NCORES * a.shape[0], *a.shape[1:]) for a in out_avals]
    gdtypes = [a.dtype for a in out_avals]
    zfn = jax.jit(
        lambda: tuple(jnp.zeros(s, d) for s, d in zip(gshapes, gdtypes)),
        out_shardings=shardings,
    )
    state = {
        "sharded": sharded,
        "zfn": zfn,
        "in_names": in_names,
        "out_names": out_names,
        "out_avals": out_avals,
        "nc": nc,
        "sharding": NamedSharding(mesh, PS("core")),
    }
    _CACHE[n_steps] = state
    return state


_POOL = None


def run(inputs: dict, n_steps: int = T):
    global _POOL
    import jax

    if _POOL is None:
        from concurrent.futures import ThreadPoolExecutor

        _POOL = ThreadPoolExecutor(2)
    st = _get_exec(n_steps)
    zeros = st["zfn"]()  # async device-side zeros
    sharding = st["sharding"]
    put = lambda arr: jax.device_put(arr, sharding)
    g = _host_prep_globals(inputs, n_steps, put=put, pool=_POOL)
    outs = st["sharded"](*[g[n] for n in st["in_names"]], *zeros)
    oname_i = st["out_names"].index("out")
    o = np.asarray(outs[oname_i])  # [8*n, BG, E] int16
    o = o.reshape(NCORES, n_steps, BG, E)
    return _assemble(list(o), n_steps)


def kernel(**inputs) -> np.ndarray:
    return run(inputs, T)


# revision 35
# speedup vs baseline: 11.4842x; 1.1424x over previous
"""AttentionDecoder Trainium2 kernel: 8-way model-parallel LSTM+attention decoder.

v2 — optimized for end-to-end wall clock through the axon tunnel (~82MB/s):
  - Quantized transfer: LSTM weights int8, h_encoder/W1/W2/xseq int16,
    output int16 (~43MB/call vs 210MB fp32 baseline).
  - Attention precompute (M1T = (h_enc @ W1).T, M2 = h_enc @ W2v.T) moved
    from host numpy (was 4.3s) onto the device prologue (fp32 PE matmuls
    on exact int16 operands, scales folded into the PSUM->SBUF copies).
  - Shared tensors (W1, W2, xseq, state inits) shipped as 1/8 shards and
    AllGathered on device.
  - All layout transposes on device (PE transpose / DMA XBAR transpose);
    host prep is quantize + contiguous reshapes only.
  - Steady-state exec path caches the jitted shard_map wrapper (no per-call
    retrace) and creates donated output zeros on device.
  - Step loop matmuls in bf16 (weights already <= 8-bit precision),
    pointwise/softmax in fp32, per-step AllGathers in bf16.

Numerics validated vs reference: rel err ~2.8e-3 (tolerance 2e-2).

Layout summary (per core k):
  - Weights sharded over the 4H gate dim: core k owns gate rows
    [g*H + k*128, g*H + (k+1)*128) for g in 0..3 of both layers.
  - Activations feature-major [feat, batch]; gates matmuls are
    lhsT = activation chunk [K=128 feats, M=B], rhs = weight.T chunk.
  - Per timestep: 3 bf16 AllGathers (h0, h1, o) across the 8 cores.
  - Attention per-core batch shard: core k owns batches 8k..8k+7.
"""

import warnings

warnings.filterwarnings("ignore")

import numpy as np

VOCAB, E, H, L, B, T, S, V = 32000, 512, 1024, 2, 64, 64, 128, 1024
NCORES = 8
P = 128
BG = B // NCORES  # 8 batches per core for attention
HC = H // NCORES  # 128 hidden feats per core
GC = 4 * HC  # 512 gate rows per core

O_SCALE = 32000.0  # fixed output quant scale (tanh output, |o| <= 1)

# ---- AllGather blob layout (int16 elems, per-core contribution) ----
N_W1 = P * H  # [128, 1024] W1 row chunk
N_W2 = 2 * P * E  # [256, 512] W2.T row chunk
N_HI = P * B  # [128, 64] hidden-init feature chunk
N_OI = P * 4 * BG  # [128, 4, 8] output-init chunk
OFF_W1 = 0
OFF_W2 = OFF_W1 + N_W1
OFF_X = OFF_W2 + N_W2


def _blob_layout(n_pad):
    n_x = (n_pad // NCORES) * P * 4 * B  # int8 xseq blob, per-core elems
    off_h0 = OFF_X
    off_h1 = off_h0 + N_HI
    off_oi = off_h1 + N_HI
    nb1 = off_oi + N_OI
    return n_x, off_h0, off_h1, off_oi, nb1


_CACHE = {}


def _build(n_steps: int):
    import concourse.bass as bass
    import concourse.bacc as bacc
    import concourse.mybir as mybir
    import concourse.tile as tile
    from concourse import masks

    fp32 = mybir.dt.float32
    bf16 = mybir.dt.bfloat16
    i16 = mybir.dt.int16
    i8 = mybir.dt.int8
    AF = mybir.ActivationFunctionType
    AX = mybir.AxisListType

    n_pad = ((n_steps + NCORES - 1) // NCORES) * NCORES
    n_x, off_h0, off_h1, off_oi, nb1 = _blob_layout(n_pad)

    nc = bacc.Bacc("TRN2", target_bir_lowering=False, debug=False, num_devices=NCORES)

    # ---- DRAM I/O ----
    d_blob = nc.dram_tensor("blob", [nb1], i16, kind="ExternalInput")
    d_xblob = nc.dram_tensor("xblob", [n_x], i8, kind="ExternalInput")
    d_lw = nc.dram_tensor("lw", [L, GC, 2048], i8, kind="ExternalInput")
    d_henc = nc.dram_tensor("henc", [BG, S, V], i8, kind="ExternalInput")
    d_c1r = nc.dram_tensor("c1r", [BG, S], fp32, kind="ExternalInput")
    d_gb = nc.dram_tensor("gb", [3, GC], fp32, kind="ExternalInput")
    d_sc = nc.dram_tensor("sc", [P, 8], fp32, kind="ExternalInput")
    d_out = nc.dram_tensor("out", [n_steps, BG, E], i16, kind="ExternalOutput")

    RG = [list(range(NCORES))]

    with tile.TileContext(nc) as tc:
        import contextlib

        ctx = contextlib.ExitStack()
        with ctx:
            wpool = ctx.enter_context(tc.tile_pool(name="weights", bufs=1))
            spool = ctx.enter_context(tc.tile_pool(name="state", bufs=1))
            propool = ctx.enter_context(tc.tile_pool(name="pro", bufs=1))
            xpool = ctx.enter_context(tc.tile_pool(name="x", bufs=2))
            tpool = ctx.enter_context(tc.tile_pool(name="tmp", bufs=2))
            ppool = ctx.enter_context(tc.tile_pool(name="psum", bufs=2, space="PSUM"))
            p1pool = ctx.enter_context(tc.tile_pool(name="psum1", bufs=1, space="PSUM"))
            dpool = ctx.enter_context(tc.tile_pool(name="dram", bufs=2, space="DRAM"))
            d1pool = ctx.enter_context(tc.tile_pool(name="dram1", bufs=1, space="DRAM"))

            # ---- persistent SBUF tiles ----
            w0T = wpool.tile([P, 16, GC], bf16, tag="w0T")
            w1T = wpool.tile([P, 16, GC], bf16, tag="w1T")
            gbb = wpool.tile([1, 3 * GC], bf16, tag="gbb")
            m1t = wpool.tile([P, BG, 8, S], bf16, tag="m1t")
            c1t = wpool.tile([P, 2, S], fp32, tag="c1t")
            m2s = wpool.tile([P, BG, E], bf16, tag="m2s")
            w2hb = wpool.tile([P, 8, E], bf16, tag="w2hb")
            ident = wpool.tile([P, P], fp32, tag="ident")
            ones = wpool.tile([1, B], bf16, tag="ones")
            sc = wpool.tile([P, 8], fp32, tag="sc")

            h0f = [
                spool.tile([P, NCORES * B], bf16, tag=f"h0f{i}", name=f"h0f{i}")
                for i in range(2)
            ]
            h1f = [
                spool.tile([P, NCORES * B], bf16, tag=f"h1f{i}", name=f"h1f{i}")
                for i in range(2)
            ]
            of = [
                spool.tile([P, NCORES * 4 * BG], bf16, tag=f"of{i}", name=f"of{i}")
                for i in range(2)
            ]
            c0 = spool.tile([B, HC], fp32, tag="c0")
            c1 = spool.tile([B, HC], fp32, tag="c1")
            h1my = spool.tile([P, 8, BG], bf16, tag="h1my")

            # ---- prologue transients ----
            w1f = propool.tile([P, 8, H], fp32, tag="w1f")
            w2vf = propool.tile([P, 8, E], fp32, tag="w2vf")
            lw8 = propool.tile([P, 4, 2048], i8, tag="lw8")
            castbuf = propool.tile([P, 2048], fp32, tag="castbuf")
            st16 = propool.tile([P, H], i16, tag="st16")
            he8 = propool.tile([P, V], i8, tag="he8")
            h16f = propool.tile([P, 8, S], fp32, tag="h16f")
            his16 = propool.tile([P, NCORES, B], i16, tag="his16")
            ois16 = propool.tile([P, NCORES * 4 * BG], i16, tag="ois16")

            # ---- basics ----
            nc.sync.dma_start(sc[:], d_sc[:])
            gbf = propool.tile([1, 3 * GC], fp32, tag="gbf")
            nc.sync.dma_start(
                gbf[:], d_gb[:].rearrange("l g -> (l g)").rearrange("(a x) -> a x", a=1)
            )
            nc.vector.tensor_copy(gbb[:], gbf[:])
            nc.vector.memset(ones[:], 1.0)
            masks.make_identity(nc, ident[:])
            nc.vector.memset(c0[:], 0.0)
            nc.vector.memset(c1[:], 0.0)

            pid = nc.vector.partition_id()

            # ---- blob AllGathers (int16 params + int8 xseq) ----
            agi = d1pool.tile([nb1], i16, tag="agi")
            ago = d1pool.tile([NCORES, nb1], i16, tag="ago")
            nc.sync.dma_start(agi[:], d_blob[:])
            nc.gpsimd.collective_compute(
                "AllGather",
                mybir.AluOpType.bypass,
                replica_groups=RG,
                ins=[agi.opt()],
                outs=[ago.opt()],
            )
            agxi = d1pool.tile([n_x], i8, tag="agxi")
            agxo = d1pool.tile([NCORES, n_x], i8, tag="agxo")
            nc.scalar.dma_start(agxi[:], d_xblob[:])
            nc.gpsimd.collective_compute(
                "AllGather",
                mybir.AluOpType.bypass,
                replica_groups=RG,
                ins=[agxi.opt()],
                outs=[agxo.opt()],
            )

            # ---- W1 -> w1f fp32 [p, vc, h] (int-valued) ----
            for vc in range(NCORES):
                nc.sync.dma_start(
                    st16[:],
                    ago[vc, OFF_W1 : OFF_W1 + N_W1].rearrange("(p h) -> p h", p=P),
                )
                nc.vector.tensor_copy(w1f[:, vc, :], st16[:])

            # ---- W2.T chunks: vc 0..7 -> w2vf fp32 int-valued;
            #      hc 0..7 -> w2hb bf16 real-valued (scale s_w2) ----
            for rc in range(16):
                k, half = rc // 2, rc % 2
                src = ago[
                    k, OFF_W2 + half * P * E : OFF_W2 + (half + 1) * P * E
                ].rearrange("(p e) -> p e", p=P)
                nc.scalar.dma_start(st16[:, 0:E], src)
                if rc < 8:
                    nc.vector.tensor_copy(w2vf[:, rc, :], st16[:, 0:E])
                else:
                    nc.scalar.activation(
                        w2hb[:, rc - 8, :], st16[:, 0:E], AF.Copy, scale=sc[:, 6:7]
                    )

            # ---- LSTM weights: int8 -> cast -> PE transpose -> scaled bf16 ----
            for l in range(L):
                wT = w0T if l == 0 else w1T
                nc.sync.dma_start(
                    lw8[:], d_lw[l].rearrange("(c p) k -> p c k", p=P)
                )
                for c in range(4):
                    nc.vector.tensor_copy(castbuf[:], lw8[:, c, :])
                    for kb in range(16):
                        ptw = ppool.tile([P, 2, GC], fp32, tag="pg", name=f"ptw{l}_{c}_{kb}")
                        nc.tensor.transpose(
                            ptw[:, 0, 0:P],
                            castbuf[:, kb * P : (kb + 1) * P],
                            ident[:],
                        )
                        nc.scalar.activation(
                            wT[:, kb, c * P : (c + 1) * P],
                            ptw[:, 0, 0:P],
                            AF.Copy,
                            scale=sc[:, 0:1],
                        )

            # ---- h_enc (int8): cast + PE transpose + m1t/m2s (scales folded) ----
            for j in range(BG):
                nc.sync.dma_start(he8[:], d_henc[j])
                nc.vector.tensor_copy(castbuf[:, 0:V], he8[:])
                for vc in range(8):
                    pht = ppool.tile([P, 2, GC], fp32, tag="pg", name=f"pht{j}_{vc}")
                    nc.tensor.transpose(
                        pht[:, 0, 0:P], castbuf[:, vc * P : (vc + 1) * P], ident[:]
                    )
                    nc.vector.tensor_copy(h16f[:, vc, :], pht[:, 0, 0:P])
                for kt in range(8):
                    pm = ppool.tile([P, 2, GC], fp32, tag="pg", name=f"pm{j}_{kt}")
                    for vc in range(8):
                        nc.tensor.matmul(
                            pm[:, 0, 0:S],
                            w1f[:, vc, kt * P : (kt + 1) * P],
                            h16f[:, vc, :],
                            start=(vc == 0),
                            stop=(vc == 7),
                        )
                    nc.scalar.activation(
                        m1t[:, j, kt, :], pm[:, 0, 0:S], AF.Copy, scale=sc[:, 4:5]
                    )
                pm2 = ppool.tile([P, 2, GC], fp32, tag="pg", name=f"pm2_{j}")
                for vc in range(8):
                    nc.tensor.matmul(
                        pm2[:, 0, :],
                        h16f[:, vc, :],
                        w2vf[:, vc, :],
                        start=(vc == 0),
                        stop=(vc == 7),
                    )
                nc.scalar.activation(
                    m2s[:, j, :], pm2[:, 0, :], AF.Copy, scale=sc[:, 5:6]
                )

            # ---- state inits from blob ----
            nc.sync.dma_start(
                his16[:], ago[:, off_h0 : off_h0 + N_HI].rearrange("k (p b) -> p k b", p=P)
            )
            nc.scalar.activation(
                h0f[1][:],
                his16[:].rearrange("p k b -> p (k b)"),
                AF.Copy,
                scale=sc[:, 2:3],
            )
            nc.sync.dma_start(
                his16[:], ago[:, off_h1 : off_h1 + N_HI].rearrange("k (p b) -> p k b", p=P)
            )
            nc.scalar.activation(
                h1f[1][:],
                his16[:].rearrange("p k b -> p (k b)"),
                AF.Copy,
                scale=sc[:, 2:3],
            )
            nc.sync.dma_start(
                ois16[:].rearrange("p (k c j) -> p k c j", k=NCORES, c=4),
                ago[:, off_oi : off_oi + N_OI].rearrange(
                    "k (p c j) -> p k c j", p=P, c=4
                ),
            )
            nc.scalar.activation(of[1][:], ois16[:], AF.Copy, scale=sc[:, 3:4])

            # ---- c1t rows ----
            nc.vector.memset(c1t[:], 0.0)
            for j in range(BG):
                nc.scalar.dma_start(
                    c1t[32 * (j % 4) : 32 * (j % 4) + 1, j // 4, :],
                    d_c1r[j].rearrange("(a s) -> a s", a=1),
                )

            def lstm_pointwise(g_sb, cst, h_out):
                """g_sb [B, 4*HC] gates i,f,g,o; updates cst, writes h_out [B,HC]."""
                gt = tpool.tile([B, HC], fp32, tag="pw_gt")
                ot = tpool.tile([B, HC], fp32, tag="pw_ot")
                ift = tpool.tile([B, 2 * HC], fp32, tag="pw_ift")
                nc.scalar.activation(ift[:], g_sb[:, 0 : 2 * HC], AF.Sigmoid)
                it, ft = ift[:, 0:HC], ift[:, HC : 2 * HC]
                nc.scalar.activation(gt[:], g_sb[:, 2 * HC : 3 * HC], AF.Tanh)
                nc.scalar.activation(ot[:], g_sb[:, 3 * HC : 4 * HC], AF.Sigmoid)
                t1 = tpool.tile([B, HC], fp32, tag="pw_t1")
                nc.vector.tensor_mul(t1[:], ft, cst[:])
                nc.vector.tensor_mul(gt[:], it, gt[:])
                nc.vector.tensor_add(cst[:], t1[:], gt[:])
                tc_ = tpool.tile([B, HC], fp32, tag="pw_tc")
                nc.scalar.activation(tc_[:], cst[:], AF.Tanh)
                nc.vector.tensor_mul(h_out[:], ot[:], tc_[:])

            def exchange(kind, src_sb, width, dst_tile):
                """Broadcast my [P,width] bf16 chunk into slot k of everyone's dst."""
                bi = dpool.tile([P, width], bf16, tag=f"agi{kind}", name=f"agi{kind}")
                bo = dpool.tile(
                    [P * NCORES, width], bf16, tag=f"ago{kind}", name=f"ago{kind}"
                )
                nc.gpsimd.dma_start(bi[:], src_sb)
                nc.gpsimd.collective_compute(
                    "AllGather",
                    mybir.AluOpType.bypass,
                    replica_groups=RG,
                    ins=[bi.opt()],
                    outs=[bo.opt()],
                )
                nc.gpsimd.dma_start(
                    dst_tile[:].rearrange("p (k w) -> p k w", k=NCORES),
                    bo[:].rearrange("(k p) w -> p k w", p=P),
                )

            x_step = P * 4 * B

            for t in range(n_steps):
                # ---- x load (int8 from AG'd xseq blob) + dequant to bf16 ----
                xi8 = xpool.tile([P, 4, B], i8, tag="xi8")
                kc, tt = t // (n_pad // NCORES), t % (n_pad // NCORES)
                nc.scalar.dma_start(
                    xi8[:],
                    agxo[kc, tt * x_step : (tt + 1) * x_step].rearrange(
                        "(p c b) -> p c b", p=P, c=4
                    ),
                )
                xt = xpool.tile([P, 4, B], bf16, tag="xt")
                nc.scalar.activation(xt[:], xi8[:], AF.Copy, scale=sc[:, 1:2])

                h0f_r = h0f[(t - 1) % 2]
                h1f_r = h1f[(t - 1) % 2]
                of_r = of[(t - 1) % 2]
                of_rv = of_r[:].rearrange("p (k c j) -> p c k j", k=NCORES, c=4)
                o4 = tpool.tile([P, 4, B], bf16, tag="o4")
                nc.vector.tensor_copy(
                    o4[:].rearrange("p c (k j) -> p c k j", k=NCORES), of_rv
                )

                # ---- gates0: bias + K = [x(4) | o(4) | h0(8)] ----
                pg0 = ppool.tile([P, 2, GC], fp32, tag="pg")
                order0 = [0, 1, 2, 3] + [8, 9, 10, 11, 12, 13, 14, 15] + [4, 5, 6, 7]
                nc.tensor.matmul(
                    pg0[0:B, 0, :], ones[:], gbb[:, 0:GC],
                    start=True, stop=False, tile_position=(0, 0),
                )
                for i, kt in enumerate(order0):
                    if kt < 4:
                        lhsT = xt[:, kt, :]
                    elif kt < 8:
                        lhsT = o4[:, kt - 4, :]
                    else:
                        lhsT = h0f_r[:, (kt - 8) * B : (kt - 7) * B]
                    hf = (i + 1) % 2
                    nc.tensor.matmul(
                        pg0[64 * hf : 64 * hf + 64, hf, :],
                        lhsT,
                        w0T[:, kt, :],
                        start=(i < 1),
                        stop=(i >= 14),
                        tile_position=(0, 64 * hf),
                    )
                g0 = tpool.tile([B, GC], fp32, tag="g0")
                nc.scalar.activation(g0[:], pg0[0:64, 0, :], AF.Copy)
                nc.vector.tensor_add(g0[:], g0[:], pg0[64:128, 1, :])
                h0m = tpool.tile([B, HC], fp32, tag="h0m")
                lstm_pointwise(g0, c0, h0m)

                # ---- transpose h0m -> [HC, B] bf16, AG -> h0f ----
                pt0 = p1pool.tile([P, P], fp32, tag="ptr", name="pt0")
                nc.tensor.transpose(pt0[:, 0:B], h0m[:], ident[0:B, 0:B])
                h0T = tpool.tile([P, B], bf16, tag="h0T")
                nc.vector.tensor_copy(h0T[:], pt0[:, 0:B])
                exchange(0, h0T[:], B, h0f[t % 2])

                # ---- gates1: bias + K = [h0(8) | h1(8)] ----
                h0f_w = h0f[t % 2]
                pg1 = ppool.tile([P, 2, GC], fp32, tag="pg")
                order1 = [8, 9, 10, 11, 12, 13, 14, 15] + [0, 1, 2, 3, 4, 5, 6, 7]
                nc.tensor.matmul(
                    pg1[0:B, 0, :], ones[:], gbb[:, GC : 2 * GC],
                    start=True, stop=False, tile_position=(0, 0),
                )
                for i, kt in enumerate(order1):
                    lhsT = (
                        h0f_w[:, kt * B : (kt + 1) * B]
                        if kt < 8
                        else h1f_r[:, (kt - 8) * B : (kt - 7) * B]
                    )
                    hf = (i + 1) % 2
                    nc.tensor.matmul(
                        pg1[64 * hf : 64 * hf + 64, hf, :],
                        lhsT,
                        w1T[:, kt, :],
                        start=(i < 1),
                        stop=(i >= 14),
                        tile_position=(0, 64 * hf),
                    )
                g1 = tpool.tile([B, GC], fp32, tag="g1")
                nc.scalar.activation(g1[:], pg1[0:64, 0, :], AF.Copy)
                nc.vector.tensor_add(g1[:], g1[:], pg1[64:128, 1, :])
                h1m = tpool.tile([B, HC], fp32, tag="h1m")
                lstm_pointwise(g1, c1, h1m)

                # ---- transpose h1m, AG -> h1f ----
                pt1 = p1pool.tile([P, P], fp32, tag="ptr", name="pt1")
                nc.tensor.transpose(pt1[:, 0:B], h1m[:], ident[0:B, 0:B])
                h1T = tpool.tile([P, B], bf16, tag="h1T")
                nc.vector.tensor_copy(h1T[:], pt1[:, 0:B])
                exchange(1, h1T[:], B, h1f[t % 2])

                # ---- select my batch columns of h1 (query) ----
                h1f_wv = h1f[t % 2][:].rearrange("p (kc b) -> p kc b", kc=8)
                nc.vector.tensor_copy(h1my[:], h1f_wv[:, :, bass.ts(pid, BG)])

                # ---- scores: per-b matvec via tile_position packing ----
                psc = p1pool.tile([P, 2, S], fp32, tag="psc")
                nc.vector.memset(psc[:], 0.0)
                for j in range(BG):
                    half, row = j // 4, 32 * (j % 4)
                    for kt in range(8):
                        nc.tensor.matmul(
                            psc[row : row + 1, half, :],
                            h1my[:, kt, j : j + 1],
                            m1t[:, j, kt, :],
                            start=(kt == 0),
                            stop=(kt == 7),
                            tile_position=(0, row),
                        )
                # ---- softmax over the two halves (garbage rows are fine) ----
                a_sb = tpool.tile([P, 2, S], fp32, tag="a_sb")
                stat = tpool.tile([P, 4], fp32, tag="stat")
                for half in range(2):
                    nc.vector.tensor_add(
                        a_sb[:, half, :], psc[:, half, :], c1t[:, half, :]
                    )
                    nm = stat[:, 2 * half : 2 * half + 1]
                    nc.vector.tensor_reduce(
                        nm, a_sb[:, half, :], axis=AX.X, op=mybir.AluOpType.max,
                        negate=True,
                    )
                    sm = stat[:, 2 * half + 1 : 2 * half + 2]
                    nc.scalar.activation(
                        a_sb[:, half, :], a_sb[:, half, :], AF.Exp, bias=nm,
                        accum_out=sm,
                    )
                    nc.vector.reciprocal(sm, sm)
                    nc.vector.tensor_scalar_mul(a_sb[:, half, :], a_sb[:, half, :], sm)

                # ---- transpose a -> columns; build block-diag lhsT (bf16) ----
                paT = p1pool.tile([P, 2, S], fp32, tag="psc", name="paT")
                nc.tensor.transpose(paT[:, 0, :], a_sb[:, 0, :], ident[:])
                nc.tensor.transpose(paT[:, 1, :], a_sb[:, 1, :], ident[:])
                abd = tpool.tile([P, BG * BG], bf16, tag="abd")
                nc.vector.memset(abd[:], 0.0)
                nc.vector.tensor_copy(
                    abd[:, 0 : BG * BG : 9].rearrange("p (a b) -> p a b", a=2),
                    paT[:].rearrange("p h (c x) -> p h c x", c=4)[:, :, :, 0:1],
                )

                # ---- z = b2 + blockdiag(a) @ M2stack + h1my.T @ W2h.T ----
                pz = p1pool.tile([BG, E], fp32, tag="pz")
                nc.tensor.matmul(
                    pz[:], ones[:, 0:BG], gbb[:, 2 * GC :], start=True, stop=False
                )
                for j in range(BG):
                    nc.tensor.matmul(
                        pz[:],
                        abd[:, j * BG : (j + 1) * BG],
                        m2s[:, j, :],
                        start=False,
                        stop=False,
                    )
                for kt in range(8):
                    nc.tensor.matmul(
                        pz[:], h1my[:, kt, :], w2hb[:, kt, :], start=False,
                        stop=(kt == 7),
                    )
                o_sb = tpool.tile([BG, E], fp32, tag="o_sb")
                nc.scalar.activation(o_sb[:], pz[:], AF.Tanh)

                # ---- write output (int16) ----
                oq = tpool.tile([BG, E], i16, tag="oq")
                nc.scalar.activation(oq[:], o_sb[:], AF.Copy, scale=O_SCALE)
                nc.scalar.dma_start(d_out[t], oq[:])

                # ---- transpose o chunks -> [P, 4, BG] bf16, AG -> of ----
                poT = p1pool.tile([P, 4, BG], fp32, tag="ptr", name="poT")
                for cchunk in range(4):
                    nc.tensor.transpose(
                        poT[:, cchunk, :],
                        o_sb[:, cchunk * P : (cchunk + 1) * P],
                        ident[0:BG, 0:BG],
                    )
                oT = tpool.tile([P, 4 * BG], bf16, tag="oT")
                nc.vector.tensor_copy(
                    oT[:].rearrange("p (c j) -> p c j", c=4), poT[:]
                )
                exchange(2, oT[:], 4 * BG, of[t % 2])

    nc.compile()
    return nc


def _scale_of(x, bits):
    m = float((1 << (bits - 1)) - 1)
    s = max(float(x.max()), -float(x.min()), 0.0)
    return s / m if s > 0 else 1.0


def _quant_to(x, s, dtype):
    buf = x * np.float32(1.0 / s)
    np.rint(buf, out=buf)
    return buf.astype(dtype)


def _host_prep_globals(inputs: dict, n_steps: int, put=None, pool=None):
    """Compute global (concatenated) device inputs; when `put` is given the
    big arrays are dispatched to the devices as soon as they are ready so the
    tunnel transfer overlaps the remaining host prep. When `pool` is given the
    independent quantize passes run on worker threads (numpy releases the
    GIL), with puts dispatched in completion order."""
    f32, i16, i8 = np.float32, np.int16, np.int8
    n_pad = ((n_steps + NCORES - 1) // NCORES) * NCORES
    n_x, off_h0, off_h1, off_oi, nb1 = _blob_layout(n_pad)
    g = {}

    tgt = np.asarray(inputs["tgt_batch"])
    h_enc = np.asarray(inputs["h_encoder"], f32)
    emb = np.asarray(inputs["emb"], f32)
    out_init = np.asarray(inputs["output_init"], f32)
    hid_init = np.asarray(inputs["hidden_init"], f32)
    W_ih = np.asarray(inputs["W_ih"], f32)
    W_hh = np.asarray(inputs["W_hh"], f32)
    b_ih = np.asarray(inputs["b_ih"], f32)
    b_hh = np.asarray(inputs["b_hh"], f32)
    W1 = np.asarray(inputs["W1"], f32)
    b1v = np.asarray(inputs["b1"], f32)
    W2 = np.asarray(inputs["W2"], f32)
    b2v = np.asarray(inputs["b2"], f32)

    def prep_lw():
        # LSTM weights: int8, gate-dim sharded, natural [rows, k_in] layout
        s_w = max(_scale_of(W_ih, 8), _scale_of(W_hh, 8))
        lw = np.empty((NCORES, L, 4, P, 2048), i8)
        lw[..., 0:1024] = _quant_to(W_ih, s_w, i8).reshape(
            2, 4, 8, P, 1024
        ).transpose(2, 0, 1, 3, 4)
        lw[..., 1024:2048] = _quant_to(W_hh, s_w, i8).reshape(
            2, 4, 8, P, 1024
        ).transpose(2, 0, 1, 3, 4)
        lw = lw.reshape(NCORES * L, GC, 2048)
        return (put(lw) if put else lw), s_w

    def prep_henc():
        # h_encoder: int8, batch-sharded, natural [S, V] layout
        s_h = _scale_of(h_enc, 8)
        hq = _quant_to(h_enc, s_h, i8).reshape(NCORES * BG, S, V)
        return (put(hq) if put else hq), s_h

    if pool is not None:
        f_lw = pool.submit(prep_lw)
        f_henc = pool.submit(prep_henc)
    else:
        g["lw"], s_w = prep_lw()
        g["henc"], s_h = prep_henc()

    # x sequence: int8, feature-major [T, P, 4, B], T-sharded
    xs = emb[tgt[:n_steps]]  # [n, B, E]
    s_x = _scale_of(xs, 8)
    xq = _quant_to(xs, s_x, i8)
    if n_pad != n_steps:
        xq = np.concatenate([xq, np.zeros((n_pad - n_steps, B, E), i8)], axis=0)
    xfold = np.ascontiguousarray(
        xq.transpose(0, 2, 1).reshape(n_pad, 4, P, B).transpose(0, 2, 1, 3)
    )  # [n_pad, P, 4, B]
    g["xblob"] = xfold.reshape(-1)

    # W1 / W2.T: int16, row-sharded
    s_w1 = _scale_of(W1, 16)
    w1s = _quant_to(W1, s_w1, i16).reshape(NCORES, P, H)
    s_w2 = _scale_of(W2, 16)
    w2s = np.ascontiguousarray(_quant_to(W2, s_w2, i16).T).reshape(NCORES, 2 * P, E)

    # state inits: int16, feature-chunk sharded
    s_hi = _scale_of(hid_init, 16)
    h0s = _quant_to(hid_init[0].T, s_hi, i16).reshape(NCORES, P, B)
    h1s = _quant_to(hid_init[1].T, s_hi, i16).reshape(NCORES, P, B)
    s_oi = _scale_of(out_init, 16)
    oi4 = _quant_to(out_init.T, s_oi, i16)
    ois = np.ascontiguousarray(
        oi4.reshape(4, P, NCORES, BG).transpose(2, 1, 0, 3)
    )  # [k, P, 4, BG]

    # blob assembly
    blob = np.empty((NCORES, nb1), i16)
    blob[:, OFF_W1 : OFF_W1 + N_W1] = w1s.reshape(NCORES, -1)
    blob[:, OFF_W2 : OFF_W2 + N_W2] = w2s.reshape(NCORES, -1)
    blob[:, off_h0 : off_h0 + N_HI] = h0s.reshape(NCORES, -1)
    blob[:, off_h1 : off_h1 + N_HI] = h1s.reshape(NCORES, -1)
    blob[:, off_oi : off_oi + N_OI] = ois.reshape(NCORES, -1)
    blob = blob.reshape(-1)
    g["blob"] = put(blob) if put else blob

    if pool is not None:
        g["lw"], s_w = f_lw.result()
        g["henc"], s_h = f_henc.result()

    # c1 rows (host fp32, exact): c1[b] = h_enc[b] @ b1
    c1v = (h_enc.reshape(-1, V) @ b1v).reshape(B, S).astype(f32)
    g["c1r"] = np.ascontiguousarray(c1v)  # [B, S] == [8*BG, S]

    # gate biases + b2
    b01 = (b_ih + b_hh).reshape(2, 4, NCORES, P).transpose(2, 0, 1, 3).reshape(
        NCORES, 2, GC
    )
    g_gb = np.empty((NCORES, 3, GC), f32)
    g_gb[:, 0:2] = b01
    g_gb[:, 2] = b2v
    g["gb"] = np.ascontiguousarray(g_gb).reshape(NCORES * 3, GC)

    # scales [P, 8]: s_w, s_x, s_hi, s_oi, s_h*s_w1, s_h*s_w2, s_w2, 0
    srow = np.array(
        [s_w, s_x, s_hi, s_oi, s_h * s_w1, s_h * s_w2, s_w2, 0.0], f32
    )
    g["sc"] = np.broadcast_to(srow, (NCORES * P, 8)).copy()
    return g


def _host_prep(inputs: dict, n_steps: int):
    """Per-core in_maps view (used by the sim harness)."""
    g = _host_prep_globals(inputs, n_steps)
    n_pad = ((n_steps + NCORES - 1) // NCORES) * NCORES
    nb1 = _blob_layout(n_pad)[-1]
    n_x = _blob_layout(n_pad)[0]
    per = {
        "blob": g["blob"].reshape(NCORES, nb1),
        "xblob": g["xblob"].reshape(NCORES, n_x),
        "lw": g["lw"].reshape(NCORES, L, GC, 2048),
        "henc": g["henc"].reshape(NCORES, BG, S, V),
        "c1r": g["c1r"].reshape(NCORES, BG, S),
        "gb": g["gb"].reshape(NCORES, 3, GC),
        "sc": g["sc"].reshape(NCORES, P, 8),
    }
    return [{name: arr[k] for name, arr in per.items()} for k in range(NCORES)]


def _assemble(outs, n_steps):
    """outs: list of per-core [n, BG, E] int16 -> [B, n, E] fp32."""
    full = np.stack(outs, axis=0)  # [k, n, BG, E]
    full = full.transpose(0, 2, 1, 3).reshape(B, n_steps, E)
    return full.astype(np.float32) * np.float32(1.0 / O_SCALE)


def _get_exec(n_steps: int):
    if n_steps in _CACHE:
        return _CACHE[n_steps]
    import jax
    import jax.numpy as jnp
    from jax.sharding import NamedSharding
    from concourse import bass2jax
    import concourse.mybir as mybir

    nc = _build(n_steps)
    bass2jax.install_neuronx_cc_hook()

    partition_name = nc.partition_id_tensor.name if nc.partition_id_tensor else None
    in_names, out_names, out_avals = [], [], []
    for alloc in nc.m.functions[0].allocations:
        if not isinstance(alloc, mybir.MemoryLocationSet):
            continue
        name = alloc.memorylocations[0].name
        if alloc.kind == "ExternalInput":
            if name != partition_name:
                in_names.append(name)
        elif alloc.kind == "ExternalOutput":
            out_names.append(name)
            out_avals.append(
                jax.core.ShapedArray(
                    tuple(alloc.tensor_shape), mybir.dt.np(alloc.dtype)
                )
            )
    n_params = len(in_names)
    all_names = list(in_names) + list(out_names)
    if partition_name is not None:
        all_names.append(partition_name)

    def _body(*args):
        operands = list(args)
        if partition_name is not None:
            operands.append(bass2jax.partition_id_tensor())
        outs = bass2jax._bass_exec_p.bind(
            *operands,
            out_avals=tuple(out_avals),
            in_names=tuple(all_names),
            out_names=tuple(out_names),
            lowering_input_output_aliases=(),
            sim_require_finite=True,
            sim_require_nnan=True,
            nc=nc,
        )
        return tuple(outs)

    devices = jax.devices()[:NCORES]
    mesh = bass2jax.Mesh(np.asarray(devices), ("core",))
    PS = bass2jax.PartitionSpec
    in_specs = (PS("core"),) * (n_params + len(out_names))
    out_specs = (PS("core"),) * len(out_names)
    donate = tuple(range(n_params, n_params + len(out_names)))
    sharded = jax.jit(
        bass2jax.shard_map(
            _body, mesh=mesh, in_specs=in_specs, out_specs=out_specs, check_rep=False
        ),
        donate_argnums=donate,
        keep_unused=True,
    )
    shardings = tuple(NamedSharding(mesh, PS("core")) for _ in out_avals)
    gshapes = [(NCORES * a.shape[0], *a.shape[1:]) for a in out_avals]
    gdtypes = [a.dtype for a in out_avals]
    zfn = jax.jit(
        lambda: tuple(jnp.zeros(s, d) for s, d in zip(gshapes, gdtypes)),
        out_shardings=shardings,
    )
    state = {
        "sharded": sharded,
        "zfn": zfn,
        "in_names": in_names,
        "out_names": out_names,
        "out_avals": out_avals,
        "nc": nc,
        "sharding": NamedSharding(mesh, PS("core")),
    }
    _CACHE[n_steps] = state
    return state


_POOL = None


def run(inputs: dict, n_steps: int = T):
    global _POOL
    import jax

    if _POOL is None:
        from concurrent.futures import ThreadPoolExecutor

        _POOL = ThreadPoolExecutor(2)
    st = _get_exec(n_steps)
    zeros = st["zfn"]()  # async device-side zeros
    sharding = st["sharding"]
    put = lambda arr: jax.device_put(arr, sharding)
    g = _host_prep_globals(inputs, n_steps, put=put, pool=_POOL)
    outs = st["sharded"](*[g[n] for n in st["in_names"]], *zeros)
    oname_i = st["out_names"].index("out")
    o = np.asarray(outs[oname_i])  # [8*n, BG, E] int16
    o = o.reshape(NCORES, n_steps, BG, E)
    return _assemble(list(o), n_steps)


def kernel(**inputs) -> np.ndarray:
    return run(inputs, T)
